# revision 31
# baseline (speedup 1.0000x reference)
"""GAT (2-layer, PyG-style) Trainium2 kernel — 8-core SPMD.

Contract: kernel(**inputs) takes FULL inputs (as produced by the problem's
setup_inputs()) and returns the FULL [N, 64] float32 output.

Strategy (dst-sharded message passing):
  - nodes partitioned into 8 contiguous shards (12500 per core); every edge is
    owned by the core that owns its dst node.  Each core sees a ROTATED node
    numbering (own shard first) so all addressing is SPMD-static.
  - Phase A (replicated): each core computes table1[n] = [h=x@W1 | a_src | a_dst]
    (bf16, 264 used cols, 768B row stride) for ALL nodes into its own HBM.
    Attention logits come free as 4 extra matmul columns (W1 is augmented).
  - Phase B: per 128-dst tile, dma_gather of table1 rows for the tile's edges
    (src rows, split into 4 int16-addressable table quarters; SWDGE descriptor
    generation at ~14ns/desc dominates, so 4 SWDGE queues spread the work).
    Per-edge a_dst needs NO gather: per-slot a_d is captured into SBUF during
    the previous phase, and adE[e] = transpose(one-hot) @ a_d_tile on the
    otherwise-idle TensorEngine.  w = exp(leaky_relu(a_s+a_d)) per edge;
    h rows scaled by w in place; one-hot [edge, dst-slot] matrix via is_equal
    against iota; the TensorEngine matmul then performs the segment softmax
    reduction (numerator and denominator in one PSUM accumulation).
    Epilogue: normalize, mean heads, bias, relu, then the layer-2 table rows
    [h2 | a_s2 | a_d2]; a_d2 captured into SBUF (valid rows only — the NaN
    rows of the last partial tile must not leak into the adE matmul).
  - AllGather of the layer-2 table shards across the 8 cores.
  - Phase C: same machinery for layer 2 -> output shard.
"""

import sys

for _p in ("/opt/trn_rl_repo",):
    if _p not in sys.path:
        sys.path.insert(0, _p)

import numpy as np

from concourse import ap_utils, bacc, bass, mybir
from concourse import tile
from concourse.bass import MemorySpace, exact_div, round_up_to_multiple
from concourse.bass_utils import run_bass_kernel_spmd

BF16 = mybir.dt.bfloat16
F32 = mybir.dt.float32
I16 = mybir.dt.int16
NP_BF16 = mybir.dt.np(BF16)

# ---------------------------------------------------------------- problem dims
N = 100000
E = 1600000
IN_DIM, HIDDEN, OUT_DIM, HEADS = 256, 128, 64, 2
NEG_SLOPE = 0.2
C1 = HEADS * HIDDEN  # 256

FULL_CFG = dict(n_cores=8, shard=12500, grp=3, n_swdge_queues=4,
                dma_scratch=65536)

P = 128
NQ = 4                      # table quarters (int16 index range)
T1C = 264                   # table1 used cols: 256 h | 2 a_s | 2 a_d
T1S = 384                   # table1 row stride in elements (768B, mult of 256B)
T2C = 66                    # table2 used cols: 64 h2 | 1 a_s2 | 1 a_d2
T2S = 128                   # table2 row stride in elements (256B)


# ================================================================ gather
def _dma_gather(gp, out_ap, in_ap, idxs_ap, num_idxs, elem_size, elem_step,
                queue_num=0):
    """bass.dma_gather with the elem%256B assert relaxed (ucode handles any
    elem size; only the row stride must be a multiple of 256B) and
    single_packet disabled (coalescing breaks past ~1k descriptors)."""
    assert idxs_ap.dtype == mybir.dt.int16
    assert in_ap.dtype == out_ap.dtype
    elem_size_bytes = elem_size * mybir.dt.size(in_ap.dtype)
    assert elem_size_bytes > 0 and elem_size_bytes % 4 == 0
    assert in_ap.space == MemorySpace.DRAM
    assert idxs_ap.space == MemorySpace.SBUF and out_ap.space == MemorySpace.SBUF
    assert ap_utils.ap_is_contiguous(out_ap.ap[1:])
    assert ap_utils.ap_is_contiguous(idxs_ap.ap[1:])
    assert in_ap.ap[-1][1] == elem_size
    assert out_ap.ap[-1][1] == elem_size
    assert out_ap.ap[0][1] * out_ap.ap[1][1] == round_up_to_multiple(num_idxs, 128)
    assert in_ap.ap[0][0] == elem_step
    stride_bytes = elem_step * mybir.dt.size(in_ap.dtype)
    stride_bytes_256 = exact_div(stride_bytes, 256)
    assert 0 < stride_bytes_256 < 256
    _in_ap = gp.lower_ap_dma(in_ap, for_custom_bir_dma=True)
    return gp.add_instruction(mybir.InstDMAGatherAnt(
        name=gp.bass.get_next_instruction_name(),
        ins=[*_in_ap, gp.lower_ap(idxs_ap),
             gp.lower_val_access(gp.to_reg(num_idxs))],
        outs=[gp.lower_ap(out_ap)],
        transpose=False, num_idxs=num_idxs, elem_size=elem_size,
        stride_bytes_256=stride_bytes_256, gen_mode=0, single_packet=False,
        queue_num=queue_num, sbuf_tokens_per_rank=0, sbuf_free_dim_per_rank=0,
        sbuf_free_dim_pad_per_rank=0, sbuf_byte_offset=0))


# ================================================================ host prep
def _balance_tiles(ld, n_tiles, shard):
    """Greedy multi-dim LPT: assign nodes to fixed-size tiles minimizing the
    max per-(tile, quarter, layer) cell size.  ld: [shard, 8] per-node loads.
    Returns node_of_row: row r (= tid*128 + slot) holds node node_of_row[r]."""
    caps = np.full(n_tiles, P, dtype=np.int64)
    caps[-1] = shard - (n_tiles - 1) * P
    loads = np.zeros((n_tiles, 8), dtype=np.int64)
    counts = np.zeros(n_tiles, dtype=np.int64)
    order = np.argsort(-ld.sum(1), kind="stable")
    assign = [[] for _ in range(n_tiles)]
    big = np.int64(1) << 40
    for o in order:
        cand = np.max(loads + ld[o], axis=1)
        cand[counts >= caps] = big
        b = int(np.argmin(cand))
        assign[b].append(o)
        loads[b] += ld[o]
        counts[b] += 1
    return np.concatenate([np.asarray(a, dtype=np.int64) for a in assign])


def _snake16(flat):
    """int16 index layout for dma_gather: logical index k sits at
    [partition k%16 (replicated x8), column k//16]."""
    cols = len(flat) // 16
    return np.tile(flat.reshape(cols, 16).T, (8, 1))


def _pack_layer(src_q, src_r, dst_local, n_tiles):
    """Group this core's edges into (tile, quarter) cells, sorted by src row
    within a cell.

    src_q: quarter of each edge's src row; src_r: row within quarter;
    dst_local: local dst id (0..shard).
    Returns (src rows, dst slots) in packed order plus per-cell counts and
    start offsets.
    """
    t_c = dst_local >> 7
    slot = (dst_local & 127).astype(np.float32)
    order = np.lexsort((src_r, src_q, t_c))
    cell = (t_c * NQ + src_q)[order]
    sr = src_r[order]
    sl = slot[order]
    counts = np.bincount(cell, minlength=n_tiles * NQ).reshape(n_tiles, NQ)
    starts = np.zeros(n_tiles * NQ + 1, dtype=np.int64)
    np.cumsum(counts.reshape(-1), out=starts[1:])
    return sr, sl, counts, starts


def _build_streams(per_core, n_tiles, grp, qch, negpad=False):
    """Build the snake16 src-index stream, the plain dstloc stream, and the
    flat (single-partition) dstloc stream for one layer."""
    n_cores = len(per_core)
    pad_idx = -1 if negpad else 0
    ch = NQ * qch
    groups = [(g, min(grp, n_tiles - g)) for g in range(0, n_tiles, grp)]
    scols = sum(gn * qch * 8 * NQ for _, gn in groups)
    lcols = n_tiles * ch
    srcq16 = np.zeros((n_cores, P, scols), dtype=np.int16)
    dstloc = np.full((n_cores, P, lcols), 255.0, dtype=NP_BF16)
    # flat per-tile edge-major dstloc: [1, n_tiles * ch * P], order within a
    # tile = (cell k = q*qch + a, slot j)
    dstlocT = np.full((n_cores, 1, n_tiles * ch * P), 255.0, dtype=NP_BF16)
    for c, (sr, sl, counts, starts) in enumerate(per_core):
        scol = 0
        for g0, gn in groups:
            ni_q = gn * qch * P
            # src stream: per quarter, tiles' cells padded to qch*128
            for q in range(NQ):
                flat = np.full(ni_q, pad_idx, dtype=np.int16)
                for ti in range(gn):
                    t = g0 + ti
                    s0 = starts[t * NQ + q]
                    cnt = counts[t, q]
                    base = ti * qch * P
                    flat[base:base + cnt] = sr[s0:s0 + cnt]
                srcq16[c, :, scol:scol + ni_q // 16] = _snake16(flat)
                scol += ni_q // 16
            # dstloc: (q, t, j) chunk order
            ni_d = gn * ch * P
            flatl = np.full(ni_d, 255.0, dtype=np.float32)
            for q in range(NQ):
                for ti in range(gn):
                    t = g0 + ti
                    s0 = starts[t * NQ + q]
                    cnt = counts[t, q]
                    base = ((q * gn) + ti) * qch * P
                    flatl[base:base + cnt] = sl[s0:s0 + cnt]
                    # edge-major layout for the K=1 broadcast matmul:
                    # tile t, cell k = q*qch + a, slot j
                    tb = t * ch * P
                    kb = q * qch * P
                    dstlocT[c, 0, tb + kb:tb + kb + cnt] = (
                        sl[s0:s0 + cnt].astype(NP_BF16))
            gbase = g0 * ch
            dstloc[c, :, gbase:gbase + gn * ch] = (
                flatl.reshape(gn * ch, P).T.astype(NP_BF16))
    return srcq16, dstloc, dstlocT


def _host_inputs(x, edge_index, W1, att_src1, att_dst1, b1, W2, att_src2,
                 att_dst2, b2, cfg):
    n_cores, shard, grp = cfg["n_cores"], cfg["shard"], cfg["grp"]
    n = x.shape[0]
    npad = ((n + 511) // 512) * 512
    assert npad % NQ == 0 and n % NQ == 0
    qs1, qs2 = npad // NQ, n // NQ
    assert qs1 <= 32768 and qs2 <= 32768 and shard <= 32768
    n_tiles = (shard + P - 1) // P

    # The PyG-style appended self-loops are NOT put into the gather streams:
    # their h rows are core-local (own table rows), so their contribution
    # w_self * h_self is added in the f32 epilogue instead.  Natural random
    # self-edges in edge_index stay in the normal path (exact multiplicity).
    src = np.asarray(edge_index[0]).astype(np.int64)
    dst = np.asarray(edge_index[1]).astype(np.int64)
    core_of = dst // shard

    # Balanced tile assignment: tile membership within a core's shard is a
    # free host-side permutation (outputs are un-permuted on the host).
    # Balancing per-(tile, quarter) cell sizes lowers qch.  Quarter of an
    # edge is permutation-invariant: own-shard layer-1 rows all fall in
    # quarter 0 (shard <= qs1), and layer-2 quarters are whole-shard aligned
    # (qs2 % shard == 0).
    balance = cfg.get("balance", True) and qs2 % shard == 0 and shard <= qs1
    edges_c, perms, perm_pos = [], [], []
    for c in range(n_cores):
        sel = core_of == c
        s_c, d_c = src[sel], dst[sel]
        o = (d_c - c * shard).astype(np.int64)
        rot = (s_c - c * shard) % n
        q1 = rot // qs1
        q2 = s_c // qs2
        edges_c.append((s_c, o, rot, q1, q2))
        if balance:
            ld = np.zeros((shard, 8), dtype=np.int64)
            for q in range(NQ):
                ld[:, q] = np.bincount(o[q1 == q], minlength=shard)
                ld[:, NQ + q] = np.bincount(o[q2 == q], minlength=shard)
            perm = _balance_tiles(ld, n_tiles, shard)
        else:
            perm = np.arange(shard, dtype=np.int64)
        pos = np.empty(shard, dtype=np.int64)
        pos[perm] = np.arange(shard)
        perms.append(perm)
        perm_pos.append(pos)
    # global permuted row of node g: pos_all[g]
    pos_all = np.concatenate(
        [c * shard + perm_pos[c] for c in range(n_cores)])

    per_core_1, per_core_2 = [], []
    maxq1 = maxq2 = 0
    for c in range(n_cores):
        s_c, o, rot, q1, q2 = edges_c[c]
        dl = perm_pos[c][o]
        # layer 1 (rotated ids; own-shard rows permuted, others unchanged)
        row1 = np.where(rot < shard, perm_pos[c][np.minimum(rot, shard - 1)],
                        rot)
        pc1 = _pack_layer(row1 // qs1, (row1 % qs1).astype(np.int16), dl,
                          n_tiles)
        per_core_1.append(pc1)
        maxq1 = max(maxq1, int(pc1[2].max()))
        # layer 2 (global permuted ids)
        row2 = pos_all[s_c]
        pc2 = _pack_layer(row2 // qs2, (row2 % qs2).astype(np.int16), dl,
                          n_tiles)
        per_core_2.append(pc2)
        maxq2 = max(maxq2, int(pc2[2].max()))

    qch1 = max(1, (maxq1 + P - 1) // P)
    qch2 = max(1, (maxq2 + P - 1) // P)
    negpad = cfg.get("negpad", False)
    s1, l1, lt1 = _build_streams(per_core_1, n_tiles, grp, qch1, negpad=negpad)
    s2, l2, lt2 = _build_streams(per_core_2, n_tiles, grp, qch2, negpad=negpad)

    x = np.asarray(x, dtype=np.float32)
    W1 = np.asarray(W1, dtype=np.float32)
    a_s1 = np.asarray(att_src1, dtype=np.float32)
    a_d1 = np.asarray(att_dst1, dtype=np.float32)
    w_as = np.einsum("khc,hc->kh", W1.reshape(IN_DIM, HEADS, HIDDEN), a_s1)
    w_ad = np.einsum("khc,hc->kh", W1.reshape(IN_DIM, HEADS, HIDDEN), a_d1)
    W1aug = np.concatenate([W1, w_as, w_ad], axis=1).astype(NP_BF16)  # [256,260]

    W2 = np.asarray(W2, dtype=np.float32)
    a_s2 = np.asarray(att_src2, dtype=np.float32).reshape(OUT_DIM)
    a_d2 = np.asarray(att_dst2, dtype=np.float32).reshape(OUT_DIM)
    W2aug = np.concatenate(
        [W2, (W2 @ a_s2)[:, None], (W2 @ a_d2)[:, None]], axis=1
    ).astype(NP_BF16)                              # [128, 66]

    b1F = np.tile(np.asarray(b1, dtype=np.float32)[None, :], (P, 1))
    b2F = np.tile(np.asarray(b2, dtype=np.float32)[None, :], (P, 1))
    iotaF = np.tile(np.arange(P, dtype=np.float32)[None, :], (P, 1)).astype(NP_BF16)
    identT = np.eye(P, dtype=np.float32).astype(NP_BF16)
    iotaColF = np.arange(P, dtype=np.float32)[:, None]  # [P, 1] f32
    onesRow = np.ones((1, P), dtype=NP_BF16)            # [1, P] bf16

    shared = dict(W1aug=W1aug, W2aug=W2aug, b1F=b1F, b2F=b2F, iotaF=iotaF,
                  identT=identT, iotaColF=iotaColF, onesRow=onesRow)
    in_maps = []
    for c in range(n_cores):
        xr = np.roll(x, -c * shard, axis=0)
        xr[:shard] = xr[:shard][perms[c]]
        xT = np.zeros((IN_DIM, npad), dtype=NP_BF16)
        xT[:, :n] = xr.T.astype(NP_BF16)
        m = dict(shared)
        m["xT"] = xT
        m["srcq1"], m["dloc1"], m["dlocT1"] = s1[c], l1[c], lt1[c]
        m["srcq2"], m["dloc2"], m["dlocT2"] = s2[c], l2[c], lt2[c]
        in_maps.append(m)
    meta = dict(qch1=qch1, qch2=qch2, npad=npad, maxq1=maxq1, maxq2=maxq2,
                perms=perms)
    return in_maps, meta


def unpermute(meta, shard, outs):
    """Un-permute per-core out_shard arrays (rows are in balanced-tile order)
    back to global node order."""
    full = np.empty((shard * len(outs), outs[0].shape[1]), outs[0].dtype)
    for c, o in enumerate(outs):
        full[c * shard + meta["perms"][c]] = o
    return full


# ================================================================ device prog
def build_program(cfg, meta):
    n_cores, shard, grp = cfg["n_cores"], cfg["shard"], cfg["grp"]
    n = cfg.get("n", N)
    npad = meta["npad"]
    qch1, qch2 = meta["qch1"], meta["qch2"]
    qs1, qs2 = npad // NQ, n // NQ
    n_tiles = (shard + P - 1) // P
    last_rows = shard - (n_tiles - 1) * P
    groups = [(g, min(grp, n_tiles - g)) for g in range(0, n_tiles, grp)]
    stop_after = cfg.get("stop_after")
    skip = cfg.get("skip", frozenset())  # timing probes: adg/srcg/mm/vec

    nsq = cfg.get("n_swdge_queues", 1)
    nc = bacc.Bacc("TRN2", target_bir_lowering=False, debug=False,
                   num_devices=n_cores, num_swdge_queues=nsq,
                   dynamic_dma_scratch_size=cfg.get("dma_scratch", 16384))

    def din(name, shape, dt):
        return nc.dram_tensor(name, shape, dt, kind="ExternalInput").ap()

    xT = din("xT", [IN_DIM, npad], BF16)
    W1aug = din("W1aug", [IN_DIM, C1 + 4], BF16)
    W2aug = din("W2aug", [HIDDEN, OUT_DIM + 2], BF16)
    b1F = din("b1F", [P, HIDDEN], F32)
    b2F = din("b2F", [P, OUT_DIM], F32)
    iotaF = din("iotaF", [P, P], BF16)
    identT = din("identT", [P, P], BF16)
    iotaColF = din("iotaColF", [P, 1], F32)
    onesRow = din("onesRow", [1, P], BF16)
    scols1 = sum(gn * qch1 * 8 * NQ for _, gn in groups)
    scols2 = sum(gn * qch2 * 8 * NQ for _, gn in groups)
    srcq1 = din("srcq1", [P, scols1], I16)
    dloc1 = din("dloc1", [P, n_tiles * NQ * qch1], BF16)
    dlocT1 = din("dlocT1", [1, n_tiles * NQ * qch1 * P], BF16)
    srcq2 = din("srcq2", [P, scols2], I16)
    dloc2 = din("dloc2", [P, n_tiles * NQ * qch2], BF16)
    dlocT2 = din("dlocT2", [1, n_tiles * NQ * qch2 * P], BF16)
    out_shard = nc.dram_tensor("out_shard", [shard, OUT_DIM], F32,
                               kind="ExternalOutput").ap()

    with tile.TileContext(nc) as tc:
        with (
            tc.tile_pool(name="dram", bufs=1, space="DRAM") as dram,
            tc.tile_pool(name="const", bufs=1) as cpool,
        ):
            # table1 split into 4 quarter tensors so layer-1 gathers of
            # quarter q only wait on phase A's writes to that quarter
            t1q = [dram.tile([qs1, T1S], BF16, name=f"t1q{q}")
                   for q in range(NQ)]
            t2shard = dram.tile([shard, T2S], BF16)
            t2full = dram.tile([shard * n_cores, T2S], BF16,
                               addr_space="Shared" if n_cores > 4 else "Local")

            w1a = cpool.tile([P, C1 + 4], BF16, tag="w1a")
            w1b = cpool.tile([P, C1 + 4], BF16, tag="w1b")
            nc.sync.dma_start(out=w1a[:, :], in_=W1aug[0:P, :])
            nc.sync.dma_start(out=w1b[:, :], in_=W1aug[P:2 * P, :])
            w2_sb = cpool.tile([P, OUT_DIM + 2], BF16, tag="w2")
            nc.sync.dma_start(out=w2_sb[:, :], in_=W2aug[:, :])
            b1_sb = cpool.tile([P, HIDDEN], F32, tag="b1")
            nc.sync.dma_start(out=b1_sb[:, :], in_=b1F[:, :])
            b2_sb = cpool.tile([P, OUT_DIM], F32, tag="b2")
            nc.sync.dma_start(out=b2_sb[:, :], in_=b2F[:, :])
            iota_sb = cpool.tile([P, P], BF16, tag="iota")
            nc.sync.dma_start(out=iota_sb[:, :], in_=iotaF[:, :])
            id_sb = cpool.tile([P, P], BF16, tag="ident")
            nc.sync.dma_start(out=id_sb[:, :], in_=identT[:, :])
            iotac_sb = cpool.tile([P, 1], F32, tag="iotac")
            nc.sync.dma_start(out=iotac_sb[:, :], in_=iotaColF[:, :])
            ones_sb = cpool.tile([1, P], BF16, tag="ones")
            nc.sync.dma_start(out=ones_sb[:, :], in_=onesRow[:, :])
            # per-slot attention logits [a_s | a_d] for the core's own dst
            # tiles, captured during the previous phase (no HBM gather needed)
            adC1 = cpool.tile([P, n_tiles, 2 * HEADS], BF16, tag="adC1")
            adC2 = cpool.tile([P, n_tiles, 2], BF16, tag="adC2")
            nc.vector.memset(adC2[:, :, :], 0.0)

            # ---------------- Phase A: table1 = [x@W1aug] for all nodes
            SLAB = 512
            with (
                tc.tile_pool(name="pa_sbuf", bufs=3) as pa,
                tc.tile_pool(name="pa_lhs", bufs=2) as pl,
                tc.tile_pool(name="pa_psum", bufs=4, space="PSUM") as pp,
            ):
                for s in range(npad // SLAB):
                    lhs0 = pl.tile([P, SLAB], BF16, tag="lhs0")
                    lhs1 = pl.tile([P, SLAB], BF16, tag="lhs1")
                    nc.sync.dma_start(out=lhs0[:, :],
                                      in_=xT[0:P, s * SLAB:(s + 1) * SLAB])
                    nc.sync.dma_start(out=lhs1[:, :],
                                      in_=xT[P:2 * P, s * SLAB:(s + 1) * SLAB])
                    for ci in range(SLAB // P):
                        rbase = s * SLAB + ci * P
                        ps = pp.tile([P, C1 + 4], F32, tag="pa_ps", space="PSUM")
                        nc.tensor.matmul(ps[:, :],
                                         lhsT=lhs0[:, ci * P:(ci + 1) * P],
                                         rhs=w1a[:, :], start=True, stop=False)
                        nc.tensor.matmul(ps[:, :],
                                         lhsT=lhs1[:, ci * P:(ci + 1) * P],
                                         rhs=w1b[:, :], start=False, stop=True)
                        stage = pa.tile([P, C1 + 4], BF16, tag="pa_stage")
                        if ci % 2 == 0:
                            nc.vector.tensor_copy(out=stage[:, :], in_=ps[:, :])
                        else:
                            nc.scalar.copy(out=stage[:, :], in_=ps[:, :])
                        tidx = rbase // P
                        if tidx < n_tiles:
                            nc.scalar.copy(out=adC1[:, tidx, :],
                                           in_=stage[:, C1:C1 + 4])
                        qi, qr = rbase // qs1, rbase % qs1
                        nc.sync.dma_start(
                            out=t1q[qi][qr:qr + P, 0:C1 + 4],
                            in_=stage[:, :])

            if stop_after != "A":
                _gat_layer(
                    nc, tc, layer=1, groups=groups, qch=qch1, n_tiles=n_tiles,
                    last_rows=last_rows, tables=t1q, tab_step=T1S,
                    qsize=qs1, hdim=C1, heads=HEADS,
                    srcq=srcq1, dlocT=dloc1, dlocF=dlocT1,
                    iota_sb=iota_sb, id_sb=id_sb, w2_sb=w2_sb,
                    b1_sb=b1_sb, b2_sb=b2_sb, adC=adC1, adC_next=adC2,
                    iotac_sb=iotac_sb, ones_sb=ones_sb, selftab=t1q[0],
                    t2shard=t2shard, out_shard=out_shard,
                    debug_out=(stop_after == "B"), skip=skip, nsq=nsq,
                )

            if stop_after in (None, "AG"):
                nc.gpsimd.collective_compute(
                    "AllGather", mybir.AluOpType.bypass,
                    replica_groups=[list(range(n_cores))],
                    ins=[t2shard[:, :]],
                    outs=[t2full[:, :]],
                )
            if stop_after is None:
                _gat_layer(
                    nc, tc, layer=2, groups=groups, qch=qch2, n_tiles=n_tiles,
                    last_rows=last_rows, tables=[t2full] * NQ, tab_step=T2S,
                    qsize=qs2, hdim=OUT_DIM, heads=1,
                    srcq=srcq2, dlocT=dloc2, dlocF=dlocT2,
                    iota_sb=iota_sb, id_sb=id_sb, w2_sb=w2_sb,
                    b1_sb=b1_sb, b2_sb=b2_sb, adC=adC2, adC_next=None,
                    iotac_sb=iotac_sb, ones_sb=ones_sb, selftab=t2shard,
                    t2shard=t2shard, out_shard=out_shard,
                    skip=skip, nsq=nsq,
                )
            elif stop_after == "A":
                with tc.tile_pool(name="dbgA", bufs=2) as pd:
                    for t in range(min(shard, 1024) // P):
                        st = pd.tile([P, T1C], BF16, tag="dbgA_t")
                        nc.sync.dma_start(out=st[:, :],
                                          in_=t1q[0][t * P:(t + 1) * P, 0:T1C])
                        sf = pd.tile([P, OUT_DIM], F32, tag="dbgA_f")
                        nc.vector.tensor_copy(out=sf[:, :], in_=st[:, 0:OUT_DIM])
                        nc.sync.dma_start(out=out_shard[t * P:(t + 1) * P, :],
                                          in_=sf[:, :])

    nc.finalize()
    return nc


def _gat_layer(nc, tc, *, layer, groups, qch, n_tiles, last_rows, tables,
               tab_step, qsize, hdim, heads, srcq, dlocT, dlocF, iota_sb,
               id_sb, w2_sb, b1_sb, b2_sb, adC, adC_next, iotac_sb, ones_sb,
               selftab, t2shard, out_shard, debug_out=False, skip=frozenset(),
               nsq=1):
    as_off = hdim            # a_s column(s) in the gathered row
    tcols = hdim + 2 * heads  # gathered row: h | a_s | a_d
    rcols = hdim + heads     # matmul rhs cols (h plus per-head w)
    ch = NQ * qch
    # per-quarter base row offset into tables[q]
    same_tab = all(t is tables[0] for t in tables)
    offs = [q * qsize if same_tab else 0 for q in range(NQ)]
    name = f"L{layer}"
    scol = 0
    BC = 512                 # K=1 broadcast matmul chunk (PSUM bank)
    n_bc = (ch * P + BC - 1) // BC
    with (
        tc.tile_pool(name=f"{name}_gath", bufs=3) as pg,
        tc.tile_pool(name=f"{name}_m", bufs=2) as pm,
        tc.tile_pool(name=f"{name}_sm", bufs=3) as psm,
        tc.tile_pool(name=f"{name}_idx", bufs=2) as pidx,
        tc.tile_pool(name=f"{name}_psum", bufs=2, space="PSUM") as pps,
        tc.tile_pool(name=f"{name}_psa", bufs=2, space="PSUM") as ppsa,
        tc.tile_pool(name=f"{name}_psb", bufs=2, space="PSUM") as ppsb,
        tc.tile_pool(name=f"{name}_pse", bufs=1, space="PSUM") as ppse,
    ):
        for g0, gn in groups:
            ni_q = gn * qch * P
            sidx = pidx.tile([P, NQ * ni_q // 16], I16, tag="sidx")
            dloc = pidx.tile([P, gn * ch], BF16, tag="dloc")
            nc.sync.dma_start(out=sidx[:, :],
                              in_=srcq[:, scol:scol + NQ * ni_q // 16])
            nc.sync.dma_start(out=dloc[:, :],
                              in_=dlocT[:, g0 * ch:(g0 + gn) * ch])
            scol += NQ * ni_q // 16

            hg = pg.tile([P, NQ, gn, qch, tcols], BF16, tag="hg")
            if "srcg" not in skip:
                for q in range(NQ):
                    _dma_gather(
                        nc.gpsimd,
                        hg[:, q, :, :, :].rearrange("p a b c -> p (a b) c"),
                        tables[q][offs[q]:offs[q] + qsize, 0:tcols],
                        sidx[:, q * ni_q // 16:(q + 1) * ni_q // 16],
                        ni_q, tcols, tab_step, queue_num=q % nsq)
            else:
                w = min(P, tcols)
                nc.vector.tensor_copy(out=hg[:, 0, 0, 0, 0:w],
                                      in_=iota_sb[:, 0:w])

            for ti in range(gn):
                tid = g0 + ti
                rows = last_rows if tid == n_tiles - 1 else P
                ht = hg[:, :, ti, :, :]          # [P, NQ, qch, tcols]
                dlt = dloc[:, :].rearrange(
                    "p (q a b) -> p q a b", q=NQ, a=gn)[:, :, ti, :]

                mt = pm.tile([P, ch, P], BF16, tag="mt")
                if "vec" not in skip:
                    iota_ap = iota_sb[:, :]
                    iota_v = bass.AP(
                        iota_ap.tensor, iota_ap.offset,
                        [list(iota_ap.ap[0]), [0, NQ], [0, qch], [1, P]])
                    nc.vector.tensor_tensor(
                        out=mt[:, :, :].rearrange("p (q a) b -> p q a b", q=NQ),
                        in0=dlt.to_broadcast([P, NQ, qch, P]),
                        in1=iota_v, op=mybir.AluOpType.is_equal)
                else:
                    nc.vector.memset(mt[:, 0, :], 0.0)

                # per-edge a_d without PE transposes: broadcast the flat
                # dstloc stream across partitions via a K=1 matmul, build the
                # TRANSPOSED one-hot mtT[slot, e] = (dloc[e] == partition)
                # with a per-partition iota scalar, then
                # adE[e, h] = sum_slot mtT[slot, e] * adC[slot, tid, h].
                adE = ppsa.tile([P, ch, heads], F32, tag="adE", space="PSUM")
                if "adg" not in skip:
                    dlf = pidx.tile([1, ch * P], BF16, tag="dlf")
                    nc.sync.dma_start(
                        out=dlf[:, :],
                        in_=dlocF[:, tid * ch * P:(tid + 1) * ch * P])
                    mtT = pm.tile([P, ch * P], BF16, tag="mtT")
                    for b in range(n_bc):
                        c0 = b * BC
                        cw = min(BC, ch * P - c0)
                        dlr = ppsb.tile([P, BC], F32, tag="dlr", space="PSUM")
                        nc.tensor.matmul(
                            dlr[:, 0:cw], lhsT=ones_sb[:, :],
                            rhs=dlf[:, c0:c0 + cw],
                            start=True, stop=True)
                        nc.vector.tensor_scalar(
                            out=mtT[:, c0:c0 + cw], in0=dlr[:, 0:cw],
                            scalar1=iotac_sb[:, 0:1], scalar2=None,
                            op0=mybir.AluOpType.is_equal)
                    for k in range(ch):
                        nc.tensor.matmul(adE[:, k, :],
                                         lhsT=mtT[:, k * P:(k + 1) * P],
                                         rhs=adC[:, tid, heads:2 * heads],
                                         start=True, stop=True)
                else:
                    nc.vector.memset(adE[:, :, :], 0.0)

                # self-loop contribution, core-local: h rows of the tile's
                # own dsts re-read contiguously, w_self from captured logits
                hs = psm.tile([P, hdim], BF16, tag="hself")
                nc.sync.dma_start(
                    out=hs[0:rows, :],
                    in_=selftab[tid * P:tid * P + rows, 0:hdim])
                wsA = psm.tile([P, heads], F32, tag="wsA")
                nc.vector.tensor_tensor(
                    out=wsA[:, :], in0=adC[:, tid, 0:heads],
                    in1=adC[:, tid, heads:2 * heads], op=mybir.AluOpType.add)
                nc.vector.scalar_tensor_tensor(
                    out=wsA[:, :], in0=wsA[:, :], scalar=NEG_SLOPE,
                    in1=wsA[:, :], op0=mybir.AluOpType.mult,
                    op1=mybir.AluOpType.max)
                wself = psm.tile([P, heads], F32, tag="wself")
                nc.scalar.activation(out=wself[:, :], in_=wsA[:, :],
                                     func=mybir.ActivationFunctionType.Exp)

                sE = psm.tile([P, NQ, qch, heads], F32, tag="sE")
                lrE = psm.tile([P, NQ, qch, heads], F32, tag="lrE")
                if "vec" not in skip:
                    nc.vector.tensor_tensor(
                        out=sE[:, :, :, :],
                        in0=ht[:, :, :, as_off:as_off + heads],
                        in1=adE[:, :, :].rearrange(
                            "p (q a) h -> p q a h", q=NQ),
                        op=mybir.AluOpType.add)
                    nc.vector.scalar_tensor_tensor(
                        out=lrE[:, :, :, :], in0=sE[:, :, :, :],
                        scalar=NEG_SLOPE, in1=sE[:, :, :, :],
                        op0=mybir.AluOpType.mult, op1=mybir.AluOpType.max)
                    nc.scalar.activation(
                        out=ht[:, :, :, as_off:as_off + heads],
                        in_=lrE[:, :, :, :],
                        func=mybir.ActivationFunctionType.Exp)

                    for q in range(NQ):
                        hv = ht[:, q, :, 0:hdim].rearrange(
                            "p a (h c) -> p a h c", h=heads)
                        wv = ht[:, q, :, as_off:as_off + heads].to_broadcast(
                            [P, qch, heads, hdim // heads])
                        nc.vector.tensor_tensor(out=hv, in0=hv, in1=wv,
                                                op=mybir.AluOpType.mult)

                ps = pps.tile([P, rcols], F32, tag="agg", space="PSUM")
                if "mm" not in skip:
                    for k in range(ch):
                        nc.tensor.matmul(ps[:, :], lhsT=mt[:, k, :],
                                         rhs=ht[:, k // qch, k % qch, 0:rcols],
                                         start=(k == 0), stop=(k == ch - 1))
                else:
                    nc.vector.memset(ps[:, :], 0.0)

                # denominators + self weight, numerators + wself*h_self
                den = psm.tile([P, heads], F32, tag="den")
                nc.vector.tensor_tensor(
                    out=den[:, :], in0=ps[:, hdim:hdim + heads],
                    in1=wself[:, :], op=mybir.AluOpType.add)
                rec = psm.tile([P, heads], F32, tag="rec")
                nc.vector.reciprocal(rec[:, :], den[:, :])
                cph = hdim // heads
                num = psm.tile([P, hdim], F32, tag="num")
                for h in range(heads):
                    nc.vector.scalar_tensor_tensor(
                        out=num[:, h * cph:(h + 1) * cph],
                        in0=hs[:, h * cph:(h + 1) * cph],
                        scalar=wself[:, h:h + 1],
                        in1=ps[:, h * cph:(h + 1) * cph],
                        op0=mybir.AluOpType.mult, op1=mybir.AluOpType.add)

                if layer == 1:
                    t0 = psm.tile([P, HIDDEN], F32, tag="t0")
                    nc.vector.tensor_scalar(
                        out=t0[:, :], in0=num[:, 0:HIDDEN],
                        scalar1=rec[:, 0:1], scalar2=None,
                        op0=mybir.AluOpType.mult)
                    nc.vector.scalar_tensor_tensor(
                        out=t0[:, :], in0=num[:, HIDDEN:2 * HIDDEN],
                        scalar=rec[:, 1:2], in1=t0[:, :],
                        op0=mybir.AluOpType.mult, op1=mybir.AluOpType.add)
                    hb = psm.tile([P, HIDDEN], F32, tag="hb")
                    nc.vector.scalar_tensor_tensor(
                        out=hb[:, :], in0=t0[:, :], scalar=0.5, in1=b1_sb[:, :],
                        op0=mybir.AluOpType.mult, op1=mybir.AluOpType.add)
                    hr = psm.tile([P, HIDDEN], BF16, tag="hr")
                    nc.scalar.activation(out=hr[:, :], in_=hb[:, :],
                                         func=mybir.ActivationFunctionType.Relu)
                    psT = ppse.tile([P, P], BF16, tag="psT", space="PSUM")
                    nc.tensor.transpose(out=psT[:, :], in_=hr[:, :],
                                        identity=id_sb[:, :])
                    hrT = psm.tile([P, P], BF16, tag="hrT")
                    nc.scalar.copy(out=hrT[:, :], in_=psT[:, :])
                    ps2 = ppse.tile([P, OUT_DIM + 2], F32, tag="ps2",
                                    space="PSUM")
                    nc.tensor.matmul(ps2[:, :], lhsT=hrT[:, :], rhs=w2_sb[:, :],
                                     start=True, stop=True)
                    t2 = psm.tile([P, OUT_DIM + 2], BF16, tag="t2")
                    nc.vector.tensor_copy(out=t2[:, :], in_=ps2[:, :])
                    nc.scalar.copy(out=adC_next[0:rows, tid, :],
                                   in_=t2[0:rows, OUT_DIM:OUT_DIM + 2])
                    nc.sync.dma_start(
                        out=t2shard[tid * P:tid * P + rows, 0:OUT_DIM + 2],
                        in_=t2[0:rows, :])
                    if debug_out:
                        dbg = psm.tile([P, OUT_DIM], F32, tag="dbg")
                        nc.vector.tensor_copy(out=dbg[:, :],
                                              in_=ps2[:, 0:OUT_DIM])
                        nc.sync.dma_start(
                            out=out_shard[tid * P:tid * P + rows, :],
                            in_=dbg[0:rows, :])
                else:
                    of = psm.tile([P, OUT_DIM], F32, tag="of")
                    nc.vector.tensor_scalar(
                        out=of[:, :], in0=num[:, 0:OUT_DIM],
                        scalar1=rec[:, 0:1], scalar2=None,
                        op0=mybir.AluOpType.mult)
                    nc.vector.tensor_tensor(
                        out=of[:, :], in0=of[:, :], in1=b2_sb[:, :],
                        op=mybir.AluOpType.add)
                    nc.sync.dma_start(
                        out=out_shard[tid * P:tid * P + rows, :],
                        in_=of[0:rows, :])


# ================================================================ entry point
def kernel(**inputs):
    cfg = dict(FULL_CFG)
    cfg["n"] = N
    in_maps, meta = _host_inputs(
        inputs["x"], inputs["edge_index"], inputs["W1"], inputs["att_src1"],
        inputs["att_dst1"], inputs["b1"], inputs["W2"], inputs["att_src2"],
        inputs["att_dst2"], inputs["b2"], cfg)
    nc = build_program(cfg, meta)
    # transient device wedges (NRT_EXEC_UNIT_UNRECOVERABLE) self-heal after a
    # few minutes; retry rather than failing the whole run
    import time as _time
    last = None
    for attempt in range(4):
        try:
            res = run_bass_kernel_spmd(
                nc, in_maps, core_ids=list(range(cfg["n_cores"])))
            break
        except Exception as exc:  # noqa: BLE001
            last = exc
            if attempt == 3:
                raise
            _time.sleep(90)
    out = unpermute(meta, cfg["shard"],
                    [res.results[c]["out_shard"]
                     for c in range(cfg["n_cores"])])
    return out.astype(np.float32)



# revision 35
# speedup vs baseline: 1.1367x; 1.1367x over previous
"""GAT (2-layer, PyG-style) Trainium2 kernel — 8-core SPMD.

Contract: kernel(**inputs) takes FULL inputs (as produced by the problem's
setup_inputs()) and returns the FULL [N, 64] float32 output.

Strategy (dst-sharded message passing):
  - nodes partitioned into 8 contiguous shards (12500 per core); every edge is
    owned by the core that owns its dst node.  Each core sees a ROTATED node
    numbering (own shard first) so all addressing is SPMD-static.
  - Phase A (replicated): each core computes table1[n] = [h=x@W1 | a_src | a_dst]
    (bf16, 264 used cols, 768B row stride) for ALL nodes into its own HBM.
    Attention logits come free as 4 extra matmul columns (W1 is augmented).
  - Phase B: per 128-dst tile, dma_gather of table1 rows for the tile's edges
    (src rows, split into 4 int16-addressable table quarters; SWDGE descriptor
    generation at ~14ns/desc dominates, so 4 SWDGE queues spread the work).
    Per-edge a_dst needs NO gather: per-slot a_d is captured into SBUF during
    the previous phase, and adE[e] = transpose(one-hot) @ a_d_tile on the
    otherwise-idle TensorEngine.  w = exp(leaky_relu(a_s+a_d)) per edge;
    h rows scaled by w in place; one-hot [edge, dst-slot] matrix via is_equal
    against iota; the TensorEngine matmul then performs the segment softmax
    reduction (numerator and denominator in one PSUM accumulation).
    Epilogue: normalize, mean heads, bias, relu, then the layer-2 table rows
    [h2 | a_s2 | a_d2]; a_d2 captured into SBUF (valid rows only — the NaN
    rows of the last partial tile must not leak into the adE matmul).
  - AllGather of the layer-2 table shards across the 8 cores.
  - Phase C: same machinery for layer 2 -> output shard.
"""

import sys

for _p in ("/opt/trn_rl_repo",):
    if _p not in sys.path:
        sys.path.insert(0, _p)

import numpy as np

from concourse import ap_utils, bacc, bass, mybir
from concourse import tile
from concourse.bass import MemorySpace, exact_div, round_up_to_multiple
from concourse.bass_utils import run_bass_kernel_spmd

BF16 = mybir.dt.bfloat16
F32 = mybir.dt.float32
I16 = mybir.dt.int16
NP_BF16 = mybir.dt.np(BF16)

# ---------------------------------------------------------------- problem dims
N = 100000
E = 1600000
IN_DIM, HIDDEN, OUT_DIM, HEADS = 256, 128, 64, 2
NEG_SLOPE = 0.2
C1 = HEADS * HIDDEN  # 256

FULL_CFG = dict(n_cores=8, shard=12500, grp=3, n_swdge_queues=4,
                dma_scratch=65536)

P = 128
NQ = 4                      # table quarters (int16 index range)
T1C = 264                   # table1 used cols: 256 h | 2 a_s | 2 a_d
T1S = 384                   # table1 row stride in elements (768B, mult of 256B)
T2C = 66                    # table2 used cols: 64 h2 | 1 a_s2 | 1 a_d2
T2S = 128                   # table2 row stride in elements (256B)


# ================================================================ gather
def _dma_gather(gp, out_ap, in_ap, idxs_ap, num_idxs, elem_size, elem_step,
                queue_num=0):
    """bass.dma_gather with the elem%256B assert relaxed (ucode handles any
    elem size; only the row stride must be a multiple of 256B) and
    single_packet disabled (coalescing breaks past ~1k descriptors)."""
    assert idxs_ap.dtype == mybir.dt.int16
    assert in_ap.dtype == out_ap.dtype
    elem_size_bytes = elem_size * mybir.dt.size(in_ap.dtype)
    assert elem_size_bytes > 0 and elem_size_bytes % 4 == 0
    assert in_ap.space == MemorySpace.DRAM
    assert idxs_ap.space == MemorySpace.SBUF and out_ap.space == MemorySpace.SBUF
    assert ap_utils.ap_is_contiguous(out_ap.ap[1:])
    assert ap_utils.ap_is_contiguous(idxs_ap.ap[1:])
    assert in_ap.ap[-1][1] == elem_size
    assert out_ap.ap[-1][1] == elem_size
    assert out_ap.ap[0][1] * out_ap.ap[1][1] == round_up_to_multiple(num_idxs, 128)
    assert in_ap.ap[0][0] == elem_step
    stride_bytes = elem_step * mybir.dt.size(in_ap.dtype)
    stride_bytes_256 = exact_div(stride_bytes, 256)
    assert 0 < stride_bytes_256 < 256
    _in_ap = gp.lower_ap_dma(in_ap, for_custom_bir_dma=True)
    return gp.add_instruction(mybir.InstDMAGatherAnt(
        name=gp.bass.get_next_instruction_name(),
        ins=[*_in_ap, gp.lower_ap(idxs_ap),
             gp.lower_val_access(gp.to_reg(num_idxs))],
        outs=[gp.lower_ap(out_ap)],
        transpose=False, num_idxs=num_idxs, elem_size=elem_size,
        stride_bytes_256=stride_bytes_256, gen_mode=0, single_packet=False,
        queue_num=queue_num, sbuf_tokens_per_rank=0, sbuf_free_dim_per_rank=0,
        sbuf_free_dim_pad_per_rank=0, sbuf_byte_offset=0))


# ================================================================ host prep
def _balance_tiles(ld, n_tiles, shard):
    """Greedy multi-dim LPT: assign nodes to fixed-size tiles minimizing the
    max per-(tile, quarter, layer) cell size.  ld: [shard, 8] per-node loads.
    Returns node_of_row: row r (= tid*128 + slot) holds node node_of_row[r]."""
    caps = np.full(n_tiles, P, dtype=np.int64)
    caps[-1] = shard - (n_tiles - 1) * P
    loads = np.zeros((n_tiles, 8), dtype=np.int64)
    counts = np.zeros(n_tiles, dtype=np.int64)
    order = np.argsort(-ld.sum(1), kind="stable")
    assign = [[] for _ in range(n_tiles)]
    big = np.int64(1) << 40
    for o in order:
        cand = np.max(loads + ld[o], axis=1)
        cand[counts >= caps] = big
        b = int(np.argmin(cand))
        assign[b].append(o)
        loads[b] += ld[o]
        counts[b] += 1
    return np.concatenate([np.asarray(a, dtype=np.int64) for a in assign])


def _snake16(flat):
    """int16 index layout for dma_gather: logical index k sits at
    [partition k%16 (replicated x8), column k//16]."""
    cols = len(flat) // 16
    return np.tile(flat.reshape(cols, 16).T, (8, 1))


def _pack_layer(src_q, src_r, dst_local, n_tiles):
    """Group this core's edges into (tile, quarter) cells, sorted by src row
    within a cell.

    src_q: quarter of each edge's src row; src_r: row within quarter;
    dst_local: local dst id (0..shard).
    Returns (src rows, dst slots) in packed order plus per-cell counts and
    start offsets.
    """
    t_c = dst_local >> 7
    slot = (dst_local & 127).astype(np.float32)
    order = np.lexsort((src_r, src_q, t_c))
    cell = (t_c * NQ + src_q)[order]
    sr = src_r[order]
    sl = slot[order]
    counts = np.bincount(cell, minlength=n_tiles * NQ).reshape(n_tiles, NQ)
    starts = np.zeros(n_tiles * NQ + 1, dtype=np.int64)
    np.cumsum(counts.reshape(-1), out=starts[1:])
    return sr, sl, counts, starts


def _build_streams(per_core, n_tiles, grp, qch, negpad=False):
    """Build the snake16 src-index stream, the plain dstloc stream, and the
    flat (single-partition) dstloc stream for one layer."""
    n_cores = len(per_core)
    pad_idx = -1 if negpad else 0
    ch = NQ * qch
    groups = [(g, min(grp, n_tiles - g)) for g in range(0, n_tiles, grp)]
    scols = sum(gn * qch * 8 * NQ for _, gn in groups)
    lcols = n_tiles * ch
    srcq16 = np.zeros((n_cores, P, scols), dtype=np.int16)
    dstloc = np.full((n_cores, P, lcols), 255.0, dtype=NP_BF16)
    # flat per-tile edge-major dstloc: [1, n_tiles * ch * P], order within a
    # tile = (cell k = q*qch + a, slot j)
    dstlocT = np.full((n_cores, 1, n_tiles * ch * P), 255.0, dtype=NP_BF16)
    for c, (sr, sl, counts, starts) in enumerate(per_core):
        scol = 0
        for g0, gn in groups:
            ni_q = gn * qch * P
            # src stream: per quarter, tiles' cells padded to qch*128
            for q in range(NQ):
                flat = np.full(ni_q, pad_idx, dtype=np.int16)
                for ti in range(gn):
                    t = g0 + ti
                    s0 = starts[t * NQ + q]
                    cnt = counts[t, q]
                    base = ti * qch * P
                    flat[base:base + cnt] = sr[s0:s0 + cnt]
                srcq16[c, :, scol:scol + ni_q // 16] = _snake16(flat)
                scol += ni_q // 16
            # dstloc: (q, t, j) chunk order
            ni_d = gn * ch * P
            flatl = np.full(ni_d, 255.0, dtype=np.float32)
            for q in range(NQ):
                for ti in range(gn):
                    t = g0 + ti
                    s0 = starts[t * NQ + q]
                    cnt = counts[t, q]
                    base = ((q * gn) + ti) * qch * P
                    flatl[base:base + cnt] = sl[s0:s0 + cnt]
                    # edge-major layout for the K=1 broadcast matmul:
                    # tile t, cell k = q*qch + a, slot j
                    tb = t * ch * P
                    kb = q * qch * P
                    dstlocT[c, 0, tb + kb:tb + kb + cnt] = (
                        sl[s0:s0 + cnt].astype(NP_BF16))
            gbase = g0 * ch
            dstloc[c, :, gbase:gbase + gn * ch] = (
                flatl.reshape(gn * ch, P).T.astype(NP_BF16))
    return srcq16, dstloc, dstlocT


def _host_inputs(x, edge_index, W1, att_src1, att_dst1, b1, W2, att_src2,
                 att_dst2, b2, cfg):
    n_cores, shard, grp = cfg["n_cores"], cfg["shard"], cfg["grp"]
    n = x.shape[0]
    npad = ((n + 511) // 512) * 512
    assert npad % NQ == 0 and n % NQ == 0
    qs1, qs2 = npad // NQ, n // NQ
    assert qs1 <= 32768 and qs2 <= 32768 and shard <= 32768
    n_tiles = (shard + P - 1) // P

    # The PyG-style appended self-loops are NOT put into the gather streams:
    # their h rows are core-local (own table rows), so their contribution
    # w_self * h_self is added in the f32 epilogue instead.  Natural random
    # self-edges in edge_index stay in the normal path (exact multiplicity).
    src = np.asarray(edge_index[0]).astype(np.int64)
    dst = np.asarray(edge_index[1]).astype(np.int64)
    core_of = dst // shard

    # Balanced tile assignment: tile membership within a core's shard is a
    # free host-side permutation (outputs are un-permuted on the host).
    # Balancing per-(tile, quarter) cell sizes lowers qch.  Quarter of an
    # edge is permutation-invariant: own-shard layer-1 rows all fall in
    # quarter 0 (shard <= qs1), and layer-2 quarters are whole-shard aligned
    # (qs2 % shard == 0).
    balance = cfg.get("balance", True) and qs2 % shard == 0 and shard <= qs1
    edges_c, perms, perm_pos = [], [], []
    for c in range(n_cores):
        sel = core_of == c
        s_c, d_c = src[sel], dst[sel]
        o = (d_c - c * shard).astype(np.int64)
        rot = (s_c - c * shard) % n
        q1 = rot // qs1
        q2 = s_c // qs2
        edges_c.append((s_c, o, rot, q1, q2))
        if balance:
            ld = np.zeros((shard, 8), dtype=np.int64)
            for q in range(NQ):
                ld[:, q] = np.bincount(o[q1 == q], minlength=shard)
                ld[:, NQ + q] = np.bincount(o[q2 == q], minlength=shard)
            perm = _balance_tiles(ld, n_tiles, shard)
        else:
            perm = np.arange(shard, dtype=np.int64)
        pos = np.empty(shard, dtype=np.int64)
        pos[perm] = np.arange(shard)
        perms.append(perm)
        perm_pos.append(pos)
    # global permuted row of node g: pos_all[g]
    pos_all = np.concatenate(
        [c * shard + perm_pos[c] for c in range(n_cores)])

    per_core_1, per_core_2 = [], []
    maxq1 = maxq2 = 0
    for c in range(n_cores):
        s_c, o, rot, q1, q2 = edges_c[c]
        dl = perm_pos[c][o]
        # layer 1 (rotated ids; own-shard rows permuted, others unchanged)
        row1 = np.where(rot < shard, perm_pos[c][np.minimum(rot, shard - 1)],
                        rot)
        pc1 = _pack_layer(row1 // qs1, (row1 % qs1).astype(np.int16), dl,
                          n_tiles)
        per_core_1.append(pc1)
        maxq1 = max(maxq1, int(pc1[2].max()))
        # layer 2 (global permuted ids)
        row2 = pos_all[s_c]
        pc2 = _pack_layer(row2 // qs2, (row2 % qs2).astype(np.int16), dl,
                          n_tiles)
        per_core_2.append(pc2)
        maxq2 = max(maxq2, int(pc2[2].max()))

    qch1 = max(1, (maxq1 + P - 1) // P)
    qch2 = max(1, (maxq2 + P - 1) // P)
    negpad = cfg.get("negpad", False)
    s1, l1, lt1 = _build_streams(per_core_1, n_tiles, grp, qch1, negpad=negpad)
    s2, l2, lt2 = _build_streams(per_core_2, n_tiles, grp, qch2, negpad=negpad)

    x = np.asarray(x, dtype=np.float32)
    W1 = np.asarray(W1, dtype=np.float32)
    a_s1 = np.asarray(att_src1, dtype=np.float32)
    a_d1 = np.asarray(att_dst1, dtype=np.float32)
    w_as = np.einsum("khc,hc->kh", W1.reshape(IN_DIM, HEADS, HIDDEN), a_s1)
    w_ad = np.einsum("khc,hc->kh", W1.reshape(IN_DIM, HEADS, HIDDEN), a_d1)
    W1aug = np.concatenate([W1, w_as, w_ad], axis=1).astype(NP_BF16)  # [256,260]

    W2 = np.asarray(W2, dtype=np.float32)
    a_s2 = np.asarray(att_src2, dtype=np.float32).reshape(OUT_DIM)
    a_d2 = np.asarray(att_dst2, dtype=np.float32).reshape(OUT_DIM)
    W2aug = np.concatenate(
        [W2, (W2 @ a_s2)[:, None], (W2 @ a_d2)[:, None]], axis=1
    ).astype(NP_BF16)                              # [128, 66]

    b1F = np.tile(np.asarray(b1, dtype=np.float32)[None, :], (P, 1))
    b2F = np.tile(np.asarray(b2, dtype=np.float32)[None, :], (P, 1))
    iotaF = np.tile(np.arange(P, dtype=np.float32)[None, :], (P, 1)).astype(NP_BF16)
    identT = np.eye(P, dtype=np.float32).astype(NP_BF16)
    iotaColF = np.arange(P, dtype=np.float32)[:, None]  # [P, 1] f32
    onesRow = np.ones((1, P), dtype=NP_BF16)            # [1, P] bf16

    shared = dict(W1aug=W1aug, W2aug=W2aug, b1F=b1F, b2F=b2F, iotaF=iotaF,
                  identT=identT, iotaColF=iotaColF, onesRow=onesRow)
    in_maps = []
    for c in range(n_cores):
        xr = np.roll(x, -c * shard, axis=0)
        xr[:shard] = xr[:shard][perms[c]]
        xT = np.zeros((IN_DIM, npad), dtype=NP_BF16)
        xT[:, :n] = xr.T.astype(NP_BF16)
        m = dict(shared)
        m["xT"] = xT
        m["srcq1"], m["dloc1"], m["dlocT1"] = s1[c], l1[c], lt1[c]
        m["srcq2"], m["dloc2"], m["dlocT2"] = s2[c], l2[c], lt2[c]
        in_maps.append(m)
    meta = dict(qch1=qch1, qch2=qch2, npad=npad, maxq1=maxq1, maxq2=maxq2,
                perms=perms)
    return in_maps, meta


def unpermute(meta, shard, outs):
    """Un-permute per-core out_shard arrays (rows are in balanced-tile order)
    back to global node order."""
    full = np.empty((shard * len(outs), outs[0].shape[1]), outs[0].dtype)
    for c, o in enumerate(outs):
        full[c * shard + meta["perms"][c]] = o
    return full


# ================================================================ device prog
def build_program(cfg, meta):
    n_cores, shard, grp = cfg["n_cores"], cfg["shard"], cfg["grp"]
    n = cfg.get("n", N)
    npad = meta["npad"]
    qch1, qch2 = meta["qch1"], meta["qch2"]
    qs1, qs2 = npad // NQ, n // NQ
    n_tiles = (shard + P - 1) // P
    last_rows = shard - (n_tiles - 1) * P
    groups = [(g, min(grp, n_tiles - g)) for g in range(0, n_tiles, grp)]
    stop_after = cfg.get("stop_after")
    skip = cfg.get("skip", frozenset())  # timing probes: adg/srcg/mm/vec

    nsq = cfg.get("n_swdge_queues", 1)
    nc = bacc.Bacc("TRN2", target_bir_lowering=False, debug=False,
                   num_devices=n_cores, num_swdge_queues=nsq,
                   dynamic_dma_scratch_size=cfg.get("dma_scratch", 16384))

    def din(name, shape, dt):
        return nc.dram_tensor(name, shape, dt, kind="ExternalInput").ap()

    xT = din("xT", [IN_DIM, npad], BF16)
    W1aug = din("W1aug", [IN_DIM, C1 + 4], BF16)
    W2aug = din("W2aug", [HIDDEN, OUT_DIM + 2], BF16)
    b1F = din("b1F", [P, HIDDEN], F32)
    b2F = din("b2F", [P, OUT_DIM], F32)
    iotaF = din("iotaF", [P, P], BF16)
    identT = din("identT", [P, P], BF16)
    iotaColF = din("iotaColF", [P, 1], F32)
    onesRow = din("onesRow", [1, P], BF16)
    scols1 = sum(gn * qch1 * 8 * NQ for _, gn in groups)
    scols2 = sum(gn * qch2 * 8 * NQ for _, gn in groups)
    srcq1 = din("srcq1", [P, scols1], I16)
    dloc1 = din("dloc1", [P, n_tiles * NQ * qch1], BF16)
    dlocT1 = din("dlocT1", [1, n_tiles * NQ * qch1 * P], BF16)
    srcq2 = din("srcq2", [P, scols2], I16)
    dloc2 = din("dloc2", [P, n_tiles * NQ * qch2], BF16)
    dlocT2 = din("dlocT2", [1, n_tiles * NQ * qch2 * P], BF16)
    out_shard = nc.dram_tensor("out_shard", [shard, OUT_DIM], F32,
                               kind="ExternalOutput").ap()

    with tile.TileContext(nc) as tc:
        with (
            tc.tile_pool(name="dram", bufs=1, space="DRAM") as dram,
            tc.tile_pool(name="const", bufs=1) as cpool,
        ):
            # table1 split into 4 quarter tensors so layer-1 gathers of
            # quarter q only wait on phase A's writes to that quarter
            t1q = [dram.tile([qs1, T1S], BF16, name=f"t1q{q}")
                   for q in range(NQ)]
            t2shard = dram.tile([shard, T2S], BF16)
            t2full = dram.tile([shard * n_cores, T2S], BF16,
                               addr_space="Shared" if n_cores > 4 else "Local")

            w1a = cpool.tile([P, C1 + 4], BF16, tag="w1a")
            w1b = cpool.tile([P, C1 + 4], BF16, tag="w1b")
            nc.sync.dma_start(out=w1a[:, :], in_=W1aug[0:P, :])
            nc.sync.dma_start(out=w1b[:, :], in_=W1aug[P:2 * P, :])
            w2_sb = cpool.tile([P, OUT_DIM + 2], BF16, tag="w2")
            nc.sync.dma_start(out=w2_sb[:, :], in_=W2aug[:, :])
            b1_sb = cpool.tile([P, HIDDEN], F32, tag="b1")
            nc.sync.dma_start(out=b1_sb[:, :], in_=b1F[:, :])
            b2_sb = cpool.tile([P, OUT_DIM], F32, tag="b2")
            nc.sync.dma_start(out=b2_sb[:, :], in_=b2F[:, :])
            iota_sb = cpool.tile([P, P], BF16, tag="iota")
            nc.sync.dma_start(out=iota_sb[:, :], in_=iotaF[:, :])
            id_sb = cpool.tile([P, P], BF16, tag="ident")
            nc.sync.dma_start(out=id_sb[:, :], in_=identT[:, :])
            iotac_sb = cpool.tile([P, 1], F32, tag="iotac")
            nc.sync.dma_start(out=iotac_sb[:, :], in_=iotaColF[:, :])
            ones_sb = cpool.tile([1, P], BF16, tag="ones")
            nc.sync.dma_start(out=ones_sb[:, :], in_=onesRow[:, :])
            # per-slot attention logits [a_s | a_d] for the core's own dst
            # tiles, captured during the previous phase (no HBM gather needed)
            adC1 = cpool.tile([P, n_tiles, 2 * HEADS], BF16, tag="adC1")
            adC2 = cpool.tile([P, n_tiles, 2], BF16, tag="adC2")
            nc.vector.memset(adC2[:, :, :], 0.0)

            # ---------------- Phase A: table1 = [x@W1aug] for all nodes
            SLAB = 512
            with (
                tc.tile_pool(name="pa_sbuf", bufs=3) as pa,
                tc.tile_pool(name="pa_lhs", bufs=2) as pl,
                tc.tile_pool(name="pa_psum", bufs=4, space="PSUM") as pp,
            ):
                for s in range(npad // SLAB):
                    lhs0 = pl.tile([P, SLAB], BF16, tag="lhs0")
                    lhs1 = pl.tile([P, SLAB], BF16, tag="lhs1")
                    nc.sync.dma_start(out=lhs0[:, :],
                                      in_=xT[0:P, s * SLAB:(s + 1) * SLAB])
                    nc.sync.dma_start(out=lhs1[:, :],
                                      in_=xT[P:2 * P, s * SLAB:(s + 1) * SLAB])
                    for ci in range(SLAB // P):
                        rbase = s * SLAB + ci * P
                        ps = pp.tile([P, C1 + 4], F32, tag="pa_ps", space="PSUM")
                        nc.tensor.matmul(ps[:, :],
                                         lhsT=lhs0[:, ci * P:(ci + 1) * P],
                                         rhs=w1a[:, :], start=True, stop=False)
                        nc.tensor.matmul(ps[:, :],
                                         lhsT=lhs1[:, ci * P:(ci + 1) * P],
                                         rhs=w1b[:, :], start=False, stop=True)
                        stage = pa.tile([P, C1 + 4], BF16, tag="pa_stage")
                        if ci % 2 == 0:
                            nc.vector.tensor_copy(out=stage[:, :], in_=ps[:, :])
                        else:
                            nc.scalar.copy(out=stage[:, :], in_=ps[:, :])
                        tidx = rbase // P
                        if tidx < n_tiles:
                            nc.scalar.copy(out=adC1[:, tidx, :],
                                           in_=stage[:, C1:C1 + 4])
                        qi, qr = rbase // qs1, rbase % qs1
                        nc.sync.dma_start(
                            out=t1q[qi][qr:qr + P, 0:C1 + 4],
                            in_=stage[:, :])

            if stop_after != "A":
                _gat_layer(
                    nc, tc, layer=1, groups=groups, qch=qch1, n_tiles=n_tiles,
                    last_rows=last_rows, tables=t1q, tab_step=T1S,
                    qsize=qs1, hdim=C1, heads=HEADS,
                    srcq=srcq1, dlocT=dloc1, dlocF=dlocT1,
                    iota_sb=iota_sb, id_sb=id_sb, w2_sb=w2_sb,
                    b1_sb=b1_sb, b2_sb=b2_sb, adC=adC1, adC_next=adC2,
                    iotac_sb=iotac_sb, ones_sb=ones_sb, selftab=t1q[0],
                    t2shard=t2shard, out_shard=out_shard,
                    debug_out=(stop_after == "B"), skip=skip, nsq=nsq,
                )

            if stop_after in (None, "AG"):
                nc.gpsimd.collective_compute(
                    "AllGather", mybir.AluOpType.bypass,
                    replica_groups=[list(range(n_cores))],
                    ins=[t2shard[:, :]],
                    outs=[t2full[:, :]],
                )
            if stop_after is None:
                _gat_layer(
                    nc, tc, layer=2, groups=groups, qch=qch2, n_tiles=n_tiles,
                    last_rows=last_rows, tables=[t2full] * NQ, tab_step=T2S,
                    qsize=qs2, hdim=OUT_DIM, heads=1,
                    srcq=srcq2, dlocT=dloc2, dlocF=dlocT2,
                    iota_sb=iota_sb, id_sb=id_sb, w2_sb=w2_sb,
                    b1_sb=b1_sb, b2_sb=b2_sb, adC=adC2, adC_next=None,
                    iotac_sb=iotac_sb, ones_sb=ones_sb, selftab=t2shard,
                    t2shard=t2shard, out_shard=out_shard,
                    skip=skip, nsq=nsq,
                )
            elif stop_after == "A":
                with tc.tile_pool(name="dbgA", bufs=2) as pd:
                    for t in range(min(shard, 1024) // P):
                        st = pd.tile([P, T1C], BF16, tag="dbgA_t")
                        nc.sync.dma_start(out=st[:, :],
                                          in_=t1q[0][t * P:(t + 1) * P, 0:T1C])
                        sf = pd.tile([P, OUT_DIM], F32, tag="dbgA_f")
                        nc.vector.tensor_copy(out=sf[:, :], in_=st[:, 0:OUT_DIM])
                        nc.sync.dma_start(out=out_shard[t * P:(t + 1) * P, :],
                                          in_=sf[:, :])

    nc.finalize()
    return nc


def _gat_layer(nc, tc, *, layer, groups, qch, n_tiles, last_rows, tables,
               tab_step, qsize, hdim, heads, srcq, dlocT, dlocF, iota_sb,
               id_sb, w2_sb, b1_sb, b2_sb, adC, adC_next, iotac_sb, ones_sb,
               selftab, t2shard, out_shard, debug_out=False, skip=frozenset(),
               nsq=1):
    as_off = hdim            # a_s column(s) in the gathered row
    # gathered row: h | a_s (a_d comes from adC); padded to 4-byte elems
    tcols = hdim + heads + (heads % 2)
    rcols = hdim + heads     # matmul rhs cols (h plus per-head w)
    ch = NQ * qch
    # per-quarter base row offset into tables[q]
    same_tab = all(t is tables[0] for t in tables)
    offs = [q * qsize if same_tab else 0 for q in range(NQ)]
    name = f"L{layer}"
    scol = 0
    BC = 512                 # K=1 broadcast matmul chunk (PSUM bank)
    n_bc = (ch * P + BC - 1) // BC
    with (
        tc.tile_pool(name=f"{name}_gath", bufs=3) as pg,
        tc.tile_pool(name=f"{name}_m", bufs=4) as pm,
        tc.tile_pool(name=f"{name}_sm", bufs=3) as psm,
        tc.tile_pool(name=f"{name}_idx", bufs=2) as pidx,
        tc.tile_pool(name=f"{name}_psum", bufs=2, space="PSUM") as pps,
        tc.tile_pool(name=f"{name}_psa", bufs=3, space="PSUM") as ppsa,
        tc.tile_pool(name=f"{name}_psb", bufs=1, space="PSUM") as ppsb,
        tc.tile_pool(name=f"{name}_pse", bufs=1, space="PSUM") as ppse,
    ):
        for g0, gn in groups:
            ni_q = gn * qch * P
            sidx = pidx.tile([P, NQ * ni_q // 16], I16, tag="sidx")
            dloc = pidx.tile([P, gn * ch], BF16, tag="dloc")
            nc.sync.dma_start(out=sidx[:, :],
                              in_=srcq[:, scol:scol + NQ * ni_q // 16])
            nc.sync.dma_start(out=dloc[:, :],
                              in_=dlocT[:, g0 * ch:(g0 + gn) * ch])
            scol += NQ * ni_q // 16

            hg = pg.tile([P, NQ, gn, qch, tcols], BF16, tag="hg")
            if "srcg" not in skip:
                for q in range(NQ):
                    _dma_gather(
                        nc.gpsimd,
                        hg[:, q, :, :, :].rearrange("p a b c -> p (a b) c"),
                        tables[q][offs[q]:offs[q] + qsize, 0:tcols],
                        sidx[:, q * ni_q // 16:(q + 1) * ni_q // 16],
                        ni_q, tcols, tab_step, queue_num=q % nsq)
            else:
                w = min(P, tcols)
                nc.vector.tensor_copy(out=hg[:, 0, 0, 0, 0:w],
                                      in_=iota_sb[:, 0:w])

            # Pass 1: gather-INDEPENDENT per-tile work (one-hots, adE, self
            # rows, w_self).  Emitted for the whole group before any
            # gather-dependent op so the in-order engine queues keep busy
            # while the gathers (and, at layer starts, phase A / the
            # AllGather) are still in flight.
            p1 = []
            for ti in range(gn):
                tid = g0 + ti
                rows = last_rows if tid == n_tiles - 1 else P
                dlt = dloc[:, :].rearrange(
                    "p (q a b) -> p q a b", q=NQ, a=gn)[:, :, ti, :]

                mt = pm.tile([P, ch, P], BF16, tag="mt")
                if "vec" not in skip:
                    iota_ap = iota_sb[:, :]
                    iota_v = bass.AP(
                        iota_ap.tensor, iota_ap.offset,
                        [list(iota_ap.ap[0]), [0, NQ], [0, qch], [1, P]])
                    nc.vector.tensor_tensor(
                        out=mt[:, :, :].rearrange("p (q a) b -> p q a b", q=NQ),
                        in0=dlt.to_broadcast([P, NQ, qch, P]),
                        in1=iota_v, op=mybir.AluOpType.is_equal)
                else:
                    nc.vector.memset(mt[:, 0, :], 0.0)

                # per-edge a_d without PE transposes: broadcast the flat
                # dstloc stream across partitions via a K=1 matmul, build the
                # TRANSPOSED one-hot mtT[slot, e] = (dloc[e] == partition)
                # with a per-partition iota scalar, then
                # adE[e, h] = sum_slot mtT[slot, e] * adC[slot, tid, h].
                adE = ppsa.tile([P, ch, heads], F32, tag="adE", space="PSUM")
                if "adg" not in skip:
                    dlf = pidx.tile([1, ch * P], BF16, tag="dlf")
                    nc.sync.dma_start(
                        out=dlf[:, :],
                        in_=dlocF[:, tid * ch * P:(tid + 1) * ch * P])
                    mtT = pm.tile([P, ch * P], BF16, tag="mtT")
                    for b in range(n_bc):
                        c0 = b * BC
                        cw = min(BC, ch * P - c0)
                        dlr = ppsb.tile([P, BC], F32, tag="dlr", space="PSUM")
                        nc.tensor.matmul(
                            dlr[:, 0:cw], lhsT=ones_sb[:, :],
                            rhs=dlf[:, c0:c0 + cw],
                            start=True, stop=True)
                        nc.vector.tensor_scalar(
                            out=mtT[:, c0:c0 + cw], in0=dlr[:, 0:cw],
                            scalar1=iotac_sb[:, 0:1], scalar2=None,
                            op0=mybir.AluOpType.is_equal)
                    for k in range(ch):
                        nc.tensor.matmul(adE[:, k, :],
                                         lhsT=mtT[:, k * P:(k + 1) * P],
                                         rhs=adC[:, tid, heads:2 * heads],
                                         start=True, stop=True)
                else:
                    nc.vector.memset(adE[:, :, :], 0.0)

                # self-loop contribution, core-local: h rows of the tile's
                # own dsts re-read contiguously, w_self from captured logits
                hs = psm.tile([P, hdim], BF16, tag="hself")
                nc.sync.dma_start(
                    out=hs[0:rows, :],
                    in_=selftab[tid * P:tid * P + rows, 0:hdim])
                wsA = psm.tile([P, heads], F32, tag="wsA")
                nc.vector.tensor_tensor(
                    out=wsA[:, :], in0=adC[:, tid, 0:heads],
                    in1=adC[:, tid, heads:2 * heads], op=mybir.AluOpType.add)
                nc.vector.scalar_tensor_tensor(
                    out=wsA[:, :], in0=wsA[:, :], scalar=NEG_SLOPE,
                    in1=wsA[:, :], op0=mybir.AluOpType.mult,
                    op1=mybir.AluOpType.max)
                wself = psm.tile([P, heads], F32, tag="wself")
                nc.scalar.activation(out=wself[:, :], in_=wsA[:, :],
                                     func=mybir.ActivationFunctionType.Exp)
                p1.append((mt, adE, hs, wself))

            # Pass 2: gather-dependent softmax, scaling, aggregation, epilogue
            for ti in range(gn):
                tid = g0 + ti
                rows = last_rows if tid == n_tiles - 1 else P
                ht = hg[:, :, ti, :, :]          # [P, NQ, qch, tcols]
                mt, adE, hs, wself = p1[ti]

                sE = psm.tile([P, NQ, qch, heads], F32, tag="sE")
                lrE = psm.tile([P, NQ, qch, heads], F32, tag="lrE")
                if "vec" not in skip:
                    nc.vector.tensor_tensor(
                        out=sE[:, :, :, :],
                        in0=ht[:, :, :, as_off:as_off + heads],
                        in1=adE[:, :, :].rearrange(
                            "p (q a) h -> p q a h", q=NQ),
                        op=mybir.AluOpType.add)
                    nc.vector.scalar_tensor_tensor(
                        out=lrE[:, :, :, :], in0=sE[:, :, :, :],
                        scalar=NEG_SLOPE, in1=sE[:, :, :, :],
                        op0=mybir.AluOpType.mult, op1=mybir.AluOpType.max)
                    nc.scalar.activation(
                        out=ht[:, :, :, as_off:as_off + heads],
                        in_=lrE[:, :, :, :],
                        func=mybir.ActivationFunctionType.Exp)

                    for q in range(NQ):
                        hv = ht[:, q, :, 0:hdim].rearrange(
                            "p a (h c) -> p a h c", h=heads)
                        wv = ht[:, q, :, as_off:as_off + heads].to_broadcast(
                            [P, qch, heads, hdim // heads])
                        nc.vector.tensor_tensor(out=hv, in0=hv, in1=wv,
                                                op=mybir.AluOpType.mult)

                ps = pps.tile([P, rcols], F32, tag="agg", space="PSUM")
                if "mm" not in skip:
                    for k in range(ch):
                        nc.tensor.matmul(ps[:, :], lhsT=mt[:, k, :],
                                         rhs=ht[:, k // qch, k % qch, 0:rcols],
                                         start=(k == 0), stop=(k == ch - 1))
                else:
                    nc.vector.memset(ps[:, :], 0.0)

                # denominators + self weight, numerators + wself*h_self
                den = psm.tile([P, heads], F32, tag="den")
                nc.vector.tensor_tensor(
                    out=den[:, :], in0=ps[:, hdim:hdim + heads],
                    in1=wself[:, :], op=mybir.AluOpType.add)
                rec = psm.tile([P, heads], F32, tag="rec")
                nc.vector.reciprocal(rec[:, :], den[:, :])
                cph = hdim // heads
                num = psm.tile([P, hdim], F32, tag="num")
                for h in range(heads):
                    nc.vector.scalar_tensor_tensor(
                        out=num[:, h * cph:(h + 1) * cph],
                        in0=hs[:, h * cph:(h + 1) * cph],
                        scalar=wself[:, h:h + 1],
                        in1=ps[:, h * cph:(h + 1) * cph],
                        op0=mybir.AluOpType.mult, op1=mybir.AluOpType.add)

                if layer == 1:
                    t0 = psm.tile([P, HIDDEN], F32, tag="t0")
                    nc.vector.tensor_scalar(
                        out=t0[:, :], in0=num[:, 0:HIDDEN],
                        scalar1=rec[:, 0:1], scalar2=None,
                        op0=mybir.AluOpType.mult)
                    nc.vector.scalar_tensor_tensor(
                        out=t0[:, :], in0=num[:, HIDDEN:2 * HIDDEN],
                        scalar=rec[:, 1:2], in1=t0[:, :],
                        op0=mybir.AluOpType.mult, op1=mybir.AluOpType.add)
                    hb = psm.tile([P, HIDDEN], F32, tag="hb")
                    nc.vector.scalar_tensor_tensor(
                        out=hb[:, :], in0=t0[:, :], scalar=0.5, in1=b1_sb[:, :],
                        op0=mybir.AluOpType.mult, op1=mybir.AluOpType.add)
                    hr = psm.tile([P, HIDDEN], BF16, tag="hr")
                    nc.scalar.activation(out=hr[:, :], in_=hb[:, :],
                                         func=mybir.ActivationFunctionType.Relu)
                    psT = ppse.tile([P, P], BF16, tag="psT", space="PSUM")
                    nc.tensor.transpose(out=psT[:, :], in_=hr[:, :],
                                        identity=id_sb[:, :])
                    hrT = psm.tile([P, P], BF16, tag="hrT")
                    nc.scalar.copy(out=hrT[:, :], in_=psT[:, :])
                    ps2 = ppse.tile([P, OUT_DIM + 2], F32, tag="ps2",
                                    space="PSUM")
                    nc.tensor.matmul(ps2[:, :], lhsT=hrT[:, :], rhs=w2_sb[:, :],
                                     start=True, stop=True)
                    t2 = psm.tile([P, OUT_DIM + 2], BF16, tag="t2")
                    nc.vector.tensor_copy(out=t2[:, :], in_=ps2[:, :])
                    nc.scalar.copy(out=adC_next[0:rows, tid, :],
                                   in_=t2[0:rows, OUT_DIM:OUT_DIM + 2])
                    nc.sync.dma_start(
                        out=t2shard[tid * P:tid * P + rows, 0:OUT_DIM + 2],
                        in_=t2[0:rows, :])
                    if debug_out:
                        dbg = psm.tile([P, OUT_DIM], F32, tag="dbg")
                        nc.vector.tensor_copy(out=dbg[:, :],
                                              in_=ps2[:, 0:OUT_DIM])
                        nc.sync.dma_start(
                            out=out_shard[tid * P:tid * P + rows, :],
                            in_=dbg[0:rows, :])
                else:
                    of = psm.tile([P, OUT_DIM], F32, tag="of")
                    nc.vector.tensor_scalar(
                        out=of[:, :], in0=num[:, 0:OUT_DIM],
                        scalar1=rec[:, 0:1], scalar2=None,
                        op0=mybir.AluOpType.mult)
                    nc.vector.tensor_tensor(
                        out=of[:, :], in0=of[:, :], in1=b2_sb[:, :],
                        op=mybir.AluOpType.add)
                    nc.sync.dma_start(
                        out=out_shard[tid * P:tid * P + rows, :],
                        in_=of[0:rows, :])


# ================================================================ entry point
def kernel(**inputs):
    cfg = dict(FULL_CFG)
    cfg["n"] = N
    in_maps, meta = _host_inputs(
        inputs["x"], inputs["edge_index"], inputs["W1"], inputs["att_src1"],
        inputs["att_dst1"], inputs["b1"], inputs["W2"], inputs["att_src2"],
        inputs["att_dst2"], inputs["b2"], cfg)
    nc = build_program(cfg, meta)
    # transient device wedges (NRT_EXEC_UNIT_UNRECOVERABLE) self-heal after a
    # few minutes; retry rather than failing the whole run
    import time as _time
    last = None
    for attempt in range(4):
        try:
            res = run_bass_kernel_spmd(
                nc, in_maps, core_ids=list(range(cfg["n_cores"])))
            break
        except Exception as exc:  # noqa: BLE001
            last = exc
            if attempt == 3:
                raise
            _time.sleep(90)
    out = unpermute(meta, cfg["shard"],
                    [res.results[c]["out_shard"]
                     for c in range(cfg["n_cores"])])
    return out.astype(np.float32)



# revision 40
# speedup vs baseline: 1.1770x; 1.0354x over previous
"""GAT (2-layer, PyG-style) Trainium2 kernel — 8-core SPMD.

Contract: kernel(**inputs) takes FULL inputs (as produced by the problem's
setup_inputs()) and returns the FULL [N, 64] float32 output.

Strategy (dst-sharded message passing):
  - nodes partitioned into 8 contiguous shards (12500 per core); every edge is
    owned by the core that owns its dst node.  Each core sees a ROTATED node
    numbering (own shard first) so all addressing is SPMD-static.
  - Phase A (replicated): each core computes table1[n] = [h=x@W1 | a_src | a_dst]
    (bf16, 264 used cols, 768B row stride) for ALL nodes into its own HBM.
    Attention logits come free as 4 extra matmul columns (W1 is augmented).
  - Phase B: per 128-dst tile, dma_gather of table1 rows for the tile's edges
    (src rows, split into 4 int16-addressable table quarters; SWDGE descriptor
    generation at ~14ns/desc dominates, so 4 SWDGE queues spread the work).
    Per-edge a_dst needs NO gather: per-slot a_d is captured into SBUF during
    the previous phase, and adE[e] = transpose(one-hot) @ a_d_tile on the
    otherwise-idle TensorEngine.  w = exp(leaky_relu(a_s+a_d)) per edge;
    h rows scaled by w in place; one-hot [edge, dst-slot] matrix via is_equal
    against iota; the TensorEngine matmul then performs the segment softmax
    reduction (numerator and denominator in one PSUM accumulation).
    Epilogue: normalize, mean heads, bias, relu, then the layer-2 table rows
    [h2 | a_s2 | a_d2]; a_d2 captured into SBUF (valid rows only — the NaN
    rows of the last partial tile must not leak into the adE matmul).
  - AllGather of the layer-2 table shards across the 8 cores.
  - Phase C: same machinery for layer 2 -> output shard.
"""

import sys

for _p in ("/opt/trn_rl_repo",):
    if _p not in sys.path:
        sys.path.insert(0, _p)

import numpy as np

from concourse import ap_utils, bacc, bass, mybir
from concourse import tile
from concourse.bass import MemorySpace, exact_div, round_up_to_multiple
from concourse.bass_utils import run_bass_kernel_spmd

BF16 = mybir.dt.bfloat16
F32 = mybir.dt.float32
I16 = mybir.dt.int16
NP_BF16 = mybir.dt.np(BF16)

# ---------------------------------------------------------------- problem dims
N = 100000
E = 1600000
IN_DIM, HIDDEN, OUT_DIM, HEADS = 256, 128, 64, 2
NEG_SLOPE = 0.2
C1 = HEADS * HIDDEN  # 256

FULL_CFG = dict(n_cores=8, shard=12500, grp=3, n_swdge_queues=4,
                dma_scratch=65536, pbufs=True)

P = 128
NQ = 4                      # table quarters (int16 index range)
T1C = 264                   # table1 used cols: 256 h | 2 a_s | 2 a_d
T1S = 384                   # table1 row stride in elements (768B, mult of 256B)
T2C = 66                    # table2 used cols: 64 h2 | 1 a_s2 | 1 a_d2
T2S = 128                   # table2 row stride in elements (256B)


# ================================================================ gather
def _dma_gather(gp, out_ap, in_ap, idxs_ap, num_idxs, elem_size, elem_step,
                queue_num=0):
    """bass.dma_gather with the elem%256B assert relaxed (ucode handles any
    elem size; only the row stride must be a multiple of 256B) and
    single_packet disabled (coalescing breaks past ~1k descriptors)."""
    assert idxs_ap.dtype == mybir.dt.int16
    assert in_ap.dtype == out_ap.dtype
    elem_size_bytes = elem_size * mybir.dt.size(in_ap.dtype)
    assert elem_size_bytes > 0 and elem_size_bytes % 4 == 0
    assert in_ap.space == MemorySpace.DRAM
    assert idxs_ap.space == MemorySpace.SBUF and out_ap.space == MemorySpace.SBUF
    assert ap_utils.ap_is_contiguous(out_ap.ap[1:])
    assert ap_utils.ap_is_contiguous(idxs_ap.ap[1:])
    assert in_ap.ap[-1][1] == elem_size
    assert out_ap.ap[-1][1] == elem_size
    assert out_ap.ap[0][1] * out_ap.ap[1][1] == round_up_to_multiple(num_idxs, 128)
    assert in_ap.ap[0][0] == elem_step
    stride_bytes = elem_step * mybir.dt.size(in_ap.dtype)
    stride_bytes_256 = exact_div(stride_bytes, 256)
    assert 0 < stride_bytes_256 < 256
    _in_ap = gp.lower_ap_dma(in_ap, for_custom_bir_dma=True)
    return gp.add_instruction(mybir.InstDMAGatherAnt(
        name=gp.bass.get_next_instruction_name(),
        ins=[*_in_ap, gp.lower_ap(idxs_ap),
             gp.lower_val_access(gp.to_reg(num_idxs))],
        outs=[gp.lower_ap(out_ap)],
        transpose=False, num_idxs=num_idxs, elem_size=elem_size,
        stride_bytes_256=stride_bytes_256, gen_mode=0, single_packet=False,
        queue_num=queue_num, sbuf_tokens_per_rank=0, sbuf_free_dim_per_rank=0,
        sbuf_free_dim_pad_per_rank=0, sbuf_byte_offset=0))


# ================================================================ host prep
def _balance_tiles(ld, n_tiles, shard):
    """Greedy multi-dim LPT: assign nodes to fixed-size tiles minimizing the
    max per-(tile, quarter, layer) cell size.  ld: [shard, 8] per-node loads.
    Returns node_of_row: row r (= tid*128 + slot) holds node node_of_row[r]."""
    caps = np.full(n_tiles, P, dtype=np.int64)
    caps[-1] = shard - (n_tiles - 1) * P
    loads = np.zeros((n_tiles, 8), dtype=np.int64)
    counts = np.zeros(n_tiles, dtype=np.int64)
    order = np.argsort(-ld.sum(1), kind="stable")
    assign = [[] for _ in range(n_tiles)]
    big = np.int64(1) << 40
    for o in order:
        cand = np.max(loads + ld[o], axis=1)
        cand[counts >= caps] = big
        b = int(np.argmin(cand))
        assign[b].append(o)
        loads[b] += ld[o]
        counts[b] += 1
    return np.concatenate([np.asarray(a, dtype=np.int64) for a in assign])


def _snake16(flat):
    """int16 index layout for dma_gather: logical index k sits at
    [partition k%16 (replicated x8), column k//16]."""
    cols = len(flat) // 16
    return np.tile(flat.reshape(cols, 16).T, (8, 1))


def _pack_layer(src_q, src_r, dst_local, n_tiles):
    """Group this core's edges into (tile, quarter) cells, sorted by src row
    within a cell.

    src_q: quarter of each edge's src row; src_r: row within quarter;
    dst_local: local dst id (0..shard).
    Returns (src rows, dst slots) in packed order plus per-cell counts and
    start offsets.
    """
    t_c = dst_local >> 7
    slot = (dst_local & 127).astype(np.float32)
    order = np.lexsort((src_r, src_q, t_c))
    cell = (t_c * NQ + src_q)[order]
    sr = src_r[order]
    sl = slot[order]
    counts = np.bincount(cell, minlength=n_tiles * NQ).reshape(n_tiles, NQ)
    starts = np.zeros(n_tiles * NQ + 1, dtype=np.int64)
    np.cumsum(counts.reshape(-1), out=starts[1:])
    return sr, sl, counts, starts


def _build_streams(per_core, n_tiles, grp, qch, negpad=False):
    """Build the snake16 src-index stream, the plain dstloc stream, and the
    flat (single-partition) dstloc stream for one layer."""
    n_cores = len(per_core)
    pad_idx = -1 if negpad else 0
    ch = NQ * qch
    groups = [(g, min(grp, n_tiles - g)) for g in range(0, n_tiles, grp)]
    scols = sum(gn * qch * 8 * NQ for _, gn in groups)
    lcols = n_tiles * ch
    srcq16 = np.zeros((n_cores, P, scols), dtype=np.int16)
    dstloc = np.full((n_cores, P, lcols), 255.0, dtype=NP_BF16)
    # flat per-tile edge-major dstloc: [1, n_tiles * ch * P], order within a
    # tile = (cell k = q*qch + a, slot j)
    dstlocT = np.full((n_cores, 1, n_tiles * ch * P), 255.0, dtype=NP_BF16)
    for c, (sr, sl, counts, starts) in enumerate(per_core):
        scol = 0
        for g0, gn in groups:
            ni_q = gn * qch * P
            # src stream: per quarter, tiles' cells padded to qch*128
            for q in range(NQ):
                flat = np.full(ni_q, pad_idx, dtype=np.int16)
                for ti in range(gn):
                    t = g0 + ti
                    s0 = starts[t * NQ + q]
                    cnt = counts[t, q]
                    base = ti * qch * P
                    flat[base:base + cnt] = sr[s0:s0 + cnt]
                srcq16[c, :, scol:scol + ni_q // 16] = _snake16(flat)
                scol += ni_q // 16
            # dstloc: (q, t, j) chunk order
            ni_d = gn * ch * P
            flatl = np.full(ni_d, 255.0, dtype=np.float32)
            for q in range(NQ):
                for ti in range(gn):
                    t = g0 + ti
                    s0 = starts[t * NQ + q]
                    cnt = counts[t, q]
                    base = ((q * gn) + ti) * qch * P
                    flatl[base:base + cnt] = sl[s0:s0 + cnt]
                    # edge-major layout for the K=1 broadcast matmul:
                    # tile t, cell k = q*qch + a, slot j
                    tb = t * ch * P
                    kb = q * qch * P
                    dstlocT[c, 0, tb + kb:tb + kb + cnt] = (
                        sl[s0:s0 + cnt].astype(NP_BF16))
            gbase = g0 * ch
            dstloc[c, :, gbase:gbase + gn * ch] = (
                flatl.reshape(gn * ch, P).T.astype(NP_BF16))
    return srcq16, dstloc, dstlocT


def _host_inputs(x, edge_index, W1, att_src1, att_dst1, b1, W2, att_src2,
                 att_dst2, b2, cfg):
    n_cores, shard, grp = cfg["n_cores"], cfg["shard"], cfg["grp"]
    n = x.shape[0]
    npad = ((n + 511) // 512) * 512
    assert npad % NQ == 0 and n % NQ == 0
    qs1, qs2 = npad // NQ, n // NQ
    assert qs1 <= 32768 and qs2 <= 32768 and shard <= 32768
    n_tiles = (shard + P - 1) // P

    # The PyG-style appended self-loops are NOT put into the gather streams:
    # their h rows are core-local (own table rows), so their contribution
    # w_self * h_self is added in the f32 epilogue instead.  Natural random
    # self-edges in edge_index stay in the normal path (exact multiplicity).
    src = np.asarray(edge_index[0]).astype(np.int64)
    dst = np.asarray(edge_index[1]).astype(np.int64)
    core_of = dst // shard

    # Balanced tile assignment: tile membership within a core's shard is a
    # free host-side permutation (outputs are un-permuted on the host).
    # Balancing per-(tile, quarter) cell sizes lowers qch.  Quarter of an
    # edge is permutation-invariant: own-shard layer-1 rows all fall in
    # quarter 0 (shard <= qs1), and layer-2 quarters are whole-shard aligned
    # (qs2 % shard == 0).
    balance = cfg.get("balance", True) and qs2 % shard == 0 and shard <= qs1
    edges_c, perms, perm_pos = [], [], []
    for c in range(n_cores):
        sel = core_of == c
        s_c, d_c = src[sel], dst[sel]
        o = (d_c - c * shard).astype(np.int64)
        rot = (s_c - c * shard) % n
        q1 = rot // qs1
        q2 = s_c // qs2
        edges_c.append((s_c, o, rot, q1, q2))
        if balance:
            ld = np.zeros((shard, 8), dtype=np.int64)
            for q in range(NQ):
                ld[:, q] = np.bincount(o[q1 == q], minlength=shard)
                ld[:, NQ + q] = np.bincount(o[q2 == q], minlength=shard)
            perm = _balance_tiles(ld, n_tiles, shard)
        else:
            perm = np.arange(shard, dtype=np.int64)
        pos = np.empty(shard, dtype=np.int64)
        pos[perm] = np.arange(shard)
        perms.append(perm)
        perm_pos.append(pos)
    # global permuted row of node g: pos_all[g]
    pos_all = np.concatenate(
        [c * shard + perm_pos[c] for c in range(n_cores)])

    per_core_1, per_core_2 = [], []
    maxq1 = maxq2 = 0
    for c in range(n_cores):
        s_c, o, rot, q1, q2 = edges_c[c]
        dl = perm_pos[c][o]
        # layer 1 (rotated ids; own-shard rows permuted, others unchanged)
        row1 = np.where(rot < shard, perm_pos[c][np.minimum(rot, shard - 1)],
                        rot)
        pc1 = _pack_layer(row1 // qs1, (row1 % qs1).astype(np.int16), dl,
                          n_tiles)
        per_core_1.append(pc1)
        maxq1 = max(maxq1, int(pc1[2].max()))
        # layer 2 (global permuted ids)
        row2 = pos_all[s_c]
        pc2 = _pack_layer(row2 // qs2, (row2 % qs2).astype(np.int16), dl,
                          n_tiles)
        per_core_2.append(pc2)
        maxq2 = max(maxq2, int(pc2[2].max()))

    qch1 = max(1, (maxq1 + P - 1) // P)
    qch2 = max(1, (maxq2 + P - 1) // P)
    negpad = cfg.get("negpad", False)
    s1, l1, lt1 = _build_streams(per_core_1, n_tiles, grp, qch1, negpad=negpad)
    s2, l2, lt2 = _build_streams(per_core_2, n_tiles, grp, qch2, negpad=negpad)

    x = np.asarray(x, dtype=np.float32)
    W1 = np.asarray(W1, dtype=np.float32)
    a_s1 = np.asarray(att_src1, dtype=np.float32)
    a_d1 = np.asarray(att_dst1, dtype=np.float32)
    w_as = np.einsum("khc,hc->kh", W1.reshape(IN_DIM, HEADS, HIDDEN), a_s1)
    w_ad = np.einsum("khc,hc->kh", W1.reshape(IN_DIM, HEADS, HIDDEN), a_d1)
    W1aug = np.concatenate([W1, w_as, w_ad], axis=1).astype(NP_BF16)  # [256,260]

    W2 = np.asarray(W2, dtype=np.float32)
    a_s2 = np.asarray(att_src2, dtype=np.float32).reshape(OUT_DIM)
    a_d2 = np.asarray(att_dst2, dtype=np.float32).reshape(OUT_DIM)
    W2aug = np.concatenate(
        [W2, (W2 @ a_s2)[:, None], (W2 @ a_d2)[:, None]], axis=1
    ).astype(NP_BF16)                              # [128, 66]

    b1F = np.tile(np.asarray(b1, dtype=np.float32)[None, :], (P, 1))
    b2F = np.tile(np.asarray(b2, dtype=np.float32)[None, :], (P, 1))
    iotaF = np.tile(np.arange(P, dtype=np.float32)[None, :], (P, 1)).astype(NP_BF16)
    identT = np.eye(P, dtype=np.float32).astype(NP_BF16)
    iotaColF = np.arange(P, dtype=np.float32)[:, None]  # [P, 1] f32
    onesRow = np.ones((1, P), dtype=NP_BF16)            # [1, P] bf16

    shared = dict(W1aug=W1aug, W2aug=W2aug, b1F=b1F, b2F=b2F, iotaF=iotaF,
                  identT=identT, iotaColF=iotaColF, onesRow=onesRow)
    in_maps = []
    for c in range(n_cores):
        xr = np.roll(x, -c * shard, axis=0)
        xr[:shard] = xr[:shard][perms[c]]
        xT = np.zeros((IN_DIM, npad), dtype=NP_BF16)
        xT[:, :n] = xr.T.astype(NP_BF16)
        m = dict(shared)
        m["xT"] = xT
        m["srcq1"], m["dloc1"], m["dlocT1"] = s1[c], l1[c], lt1[c]
        m["srcq2"], m["dloc2"], m["dlocT2"] = s2[c], l2[c], lt2[c]
        in_maps.append(m)
    meta = dict(qch1=qch1, qch2=qch2, npad=npad, maxq1=maxq1, maxq2=maxq2,
                perms=perms)
    return in_maps, meta


def unpermute(meta, shard, outs):
    """Un-permute per-core out_shard arrays (rows are in balanced-tile order)
    back to global node order."""
    full = np.empty((shard * len(outs), outs[0].shape[1]), outs[0].dtype)
    for c, o in enumerate(outs):
        full[c * shard + meta["perms"][c]] = o
    return full


# ================================================================ device prog
def build_program(cfg, meta):
    n_cores, shard, grp = cfg["n_cores"], cfg["shard"], cfg["grp"]
    n = cfg.get("n", N)
    npad = meta["npad"]
    qch1, qch2 = meta["qch1"], meta["qch2"]
    qs1, qs2 = npad // NQ, n // NQ
    n_tiles = (shard + P - 1) // P
    last_rows = shard - (n_tiles - 1) * P
    groups = [(g, min(grp, n_tiles - g)) for g in range(0, n_tiles, grp)]
    stop_after = cfg.get("stop_after")
    skip = cfg.get("skip", frozenset())  # timing probes: adg/srcg/mm/vec

    nsq = cfg.get("n_swdge_queues", 1)
    nc = bacc.Bacc("TRN2", target_bir_lowering=False, debug=False,
                   num_devices=n_cores, num_swdge_queues=nsq,
                   dynamic_dma_scratch_size=cfg.get("dma_scratch", 16384))

    def din(name, shape, dt):
        return nc.dram_tensor(name, shape, dt, kind="ExternalInput").ap()

    xT = din("xT", [IN_DIM, npad], BF16)
    W1aug = din("W1aug", [IN_DIM, C1 + 4], BF16)
    W2aug = din("W2aug", [HIDDEN, OUT_DIM + 2], BF16)
    b1F = din("b1F", [P, HIDDEN], F32)
    b2F = din("b2F", [P, OUT_DIM], F32)
    iotaF = din("iotaF", [P, P], BF16)
    identT = din("identT", [P, P], BF16)
    iotaColF = din("iotaColF", [P, 1], F32)
    onesRow = din("onesRow", [1, P], BF16)
    scols1 = sum(gn * qch1 * 8 * NQ for _, gn in groups)
    scols2 = sum(gn * qch2 * 8 * NQ for _, gn in groups)
    srcq1 = din("srcq1", [P, scols1], I16)
    dloc1 = din("dloc1", [P, n_tiles * NQ * qch1], BF16)
    dlocT1 = din("dlocT1", [1, n_tiles * NQ * qch1 * P], BF16)
    srcq2 = din("srcq2", [P, scols2], I16)
    dloc2 = din("dloc2", [P, n_tiles * NQ * qch2], BF16)
    dlocT2 = din("dlocT2", [1, n_tiles * NQ * qch2 * P], BF16)
    out_shard = nc.dram_tensor("out_shard", [shard, OUT_DIM], F32,
                               kind="ExternalOutput").ap()

    with tile.TileContext(nc) as tc:
        with (
            tc.tile_pool(name="dram", bufs=1, space="DRAM") as dram,
            tc.tile_pool(name="const", bufs=1) as cpool,
        ):
            # table1 split into 4 quarter tensors so layer-1 gathers of
            # quarter q only wait on phase A's writes to that quarter
            t1q = [dram.tile([qs1, T1S], BF16, name=f"t1q{q}")
                   for q in range(NQ)]
            t2shard = dram.tile([shard, T2S], BF16)
            t2full = dram.tile([shard * n_cores, T2S], BF16,
                               addr_space="Shared" if n_cores > 4 else "Local")

            w1a = cpool.tile([P, C1 + 4], BF16, tag="w1a")
            w1b = cpool.tile([P, C1 + 4], BF16, tag="w1b")
            nc.sync.dma_start(out=w1a[:, :], in_=W1aug[0:P, :])
            nc.sync.dma_start(out=w1b[:, :], in_=W1aug[P:2 * P, :])
            w2_sb = cpool.tile([P, OUT_DIM + 2], BF16, tag="w2")
            nc.sync.dma_start(out=w2_sb[:, :], in_=W2aug[:, :])
            b1_sb = cpool.tile([P, HIDDEN], F32, tag="b1")
            nc.sync.dma_start(out=b1_sb[:, :], in_=b1F[:, :])
            b2_sb = cpool.tile([P, OUT_DIM], F32, tag="b2")
            nc.sync.dma_start(out=b2_sb[:, :], in_=b2F[:, :])
            iota_sb = cpool.tile([P, P], BF16, tag="iota")
            nc.sync.dma_start(out=iota_sb[:, :], in_=iotaF[:, :])
            id_sb = cpool.tile([P, P], BF16, tag="ident")
            nc.sync.dma_start(out=id_sb[:, :], in_=identT[:, :])
            iotac_sb = cpool.tile([P, 1], F32, tag="iotac")
            nc.sync.dma_start(out=iotac_sb[:, :], in_=iotaColF[:, :])
            ones_sb = cpool.tile([1, P], BF16, tag="ones")
            nc.sync.dma_start(out=ones_sb[:, :], in_=onesRow[:, :])
            # per-slot attention logits [a_s | a_d] for the core's own dst
            # tiles, captured during the previous phase (no HBM gather needed)
            adC1 = cpool.tile([P, n_tiles, 2 * HEADS], BF16, tag="adC1")
            adC2 = cpool.tile([P, n_tiles, 2], BF16, tag="adC2")
            nc.vector.memset(adC2[:, :, :], 0.0)

            # ---------------- Phase A: table1 = [x@W1aug] for all nodes
            SLAB = 512
            with (
                tc.tile_pool(name="pa_sbuf", bufs=3) as pa,
                tc.tile_pool(name="pa_lhs", bufs=2) as pl,
                tc.tile_pool(name="pa_psum", bufs=4, space="PSUM") as pp,
            ):
                for s in range(npad // SLAB):
                    lhs0 = pl.tile([P, SLAB], BF16, tag="lhs0")
                    lhs1 = pl.tile([P, SLAB], BF16, tag="lhs1")
                    nc.sync.dma_start(out=lhs0[:, :],
                                      in_=xT[0:P, s * SLAB:(s + 1) * SLAB])
                    nc.sync.dma_start(out=lhs1[:, :],
                                      in_=xT[P:2 * P, s * SLAB:(s + 1) * SLAB])
                    for ci in range(SLAB // P):
                        rbase = s * SLAB + ci * P
                        ps = pp.tile([P, C1 + 4], F32, tag="pa_ps", space="PSUM")
                        nc.tensor.matmul(ps[:, :],
                                         lhsT=lhs0[:, ci * P:(ci + 1) * P],
                                         rhs=w1a[:, :], start=True, stop=False)
                        nc.tensor.matmul(ps[:, :],
                                         lhsT=lhs1[:, ci * P:(ci + 1) * P],
                                         rhs=w1b[:, :], start=False, stop=True)
                        stage = pa.tile([P, C1 + 4], BF16, tag="pa_stage")
                        if ci % 2 == 0:
                            nc.vector.tensor_copy(out=stage[:, :], in_=ps[:, :])
                        else:
                            nc.scalar.copy(out=stage[:, :], in_=ps[:, :])
                        tidx = rbase // P
                        if tidx < n_tiles:
                            nc.scalar.copy(out=adC1[:, tidx, :],
                                           in_=stage[:, C1:C1 + 4])
                        qi, qr = rbase // qs1, rbase % qs1
                        nc.sync.dma_start(
                            out=t1q[qi][qr:qr + P, 0:C1 + 4],
                            in_=stage[:, :])

            if stop_after != "A":
                _gat_layer(
                    nc, tc, layer=1, groups=groups, qch=qch1, n_tiles=n_tiles,
                    last_rows=last_rows, tables=t1q, tab_step=T1S,
                    qsize=qs1, hdim=C1, heads=HEADS,
                    srcq=srcq1, dlocT=dloc1, dlocF=dlocT1,
                    iota_sb=iota_sb, id_sb=id_sb, w2_sb=w2_sb,
                    b1_sb=b1_sb, b2_sb=b2_sb, adC=adC1, adC_next=adC2,
                    iotac_sb=iotac_sb, ones_sb=ones_sb, selftab=t1q[0],
                    t2shard=t2shard, out_shard=out_shard,
                    debug_out=(stop_after == "B"), skip=skip, nsq=nsq,
                    gmt=cfg.get("gmt", False),
                    pbufs=cfg.get("pbufs", False), pbc=cfg.get("pbc", False),
                )

            if stop_after in (None, "AG"):
                nc.gpsimd.collective_compute(
                    "AllGather", mybir.AluOpType.bypass,
                    replica_groups=[list(range(n_cores))],
                    ins=[t2shard[:, :]],
                    outs=[t2full[:, :]],
                )
            if stop_after is None:
                _gat_layer(
                    nc, tc, layer=2, groups=groups, qch=qch2, n_tiles=n_tiles,
                    last_rows=last_rows, tables=[t2full] * NQ, tab_step=T2S,
                    qsize=qs2, hdim=OUT_DIM, heads=1,
                    srcq=srcq2, dlocT=dloc2, dlocF=dlocT2,
                    iota_sb=iota_sb, id_sb=id_sb, w2_sb=w2_sb,
                    b1_sb=b1_sb, b2_sb=b2_sb, adC=adC2, adC_next=None,
                    iotac_sb=iotac_sb, ones_sb=ones_sb, selftab=t2shard,
                    t2shard=t2shard, out_shard=out_shard,
                    skip=skip, nsq=nsq, gmt=cfg.get("gmt", False),
                    pbufs=cfg.get("pbufs", False), pbc=cfg.get("pbc", False),
                )
            elif stop_after == "A":
                with tc.tile_pool(name="dbgA", bufs=2) as pd:
                    for t in range(min(shard, 1024) // P):
                        st = pd.tile([P, T1C], BF16, tag="dbgA_t")
                        nc.sync.dma_start(out=st[:, :],
                                          in_=t1q[0][t * P:(t + 1) * P, 0:T1C])
                        sf = pd.tile([P, OUT_DIM], F32, tag="dbgA_f")
                        nc.vector.tensor_copy(out=sf[:, :], in_=st[:, 0:OUT_DIM])
                        nc.sync.dma_start(out=out_shard[t * P:(t + 1) * P, :],
                                          in_=sf[:, :])

    nc.finalize()
    return nc


def _gat_layer(nc, tc, *, layer, groups, qch, n_tiles, last_rows, tables,
               tab_step, qsize, hdim, heads, srcq, dlocT, dlocF, iota_sb,
               id_sb, w2_sb, b1_sb, b2_sb, adC, adC_next, iotac_sb, ones_sb,
               selftab, t2shard, out_shard, debug_out=False, skip=frozenset(),
               nsq=1, gmt=False, pbufs=False, pbc=False):
    as_off = hdim            # a_s column(s) in the gathered row
    # gathered row: h | a_s (a_d comes from adC); padded to 4-byte elems
    tcols = hdim + heads + (heads % 2)
    rcols = hdim + heads     # matmul rhs cols (h plus per-head w)
    ch = NQ * qch
    # per-quarter base row offset into tables[q]
    same_tab = all(t is tables[0] for t in tables)
    offs = [q * qsize if same_tab else 0 for q in range(NQ)]
    name = f"L{layer}"
    scol = 0
    BC = 512                 # K=1 broadcast matmul chunk (PSUM bank)
    n_bc = (ch * P + BC - 1) // BC
    with (
        tc.tile_pool(name=f"{name}_gath", bufs=3) as pg,
        tc.tile_pool(name=f"{name}_m", bufs=4) as pm,
        tc.tile_pool(name=f"{name}_sm", bufs=3) as psm,
        tc.tile_pool(name=f"{name}_idx", bufs=2) as pidx,
        tc.tile_pool(name=f"{name}_psum", bufs=2, space="PSUM") as pps,
        tc.tile_pool(name=f"{name}_psa", bufs=2 if pbufs else 3,
                     space="PSUM") as ppsa,
        tc.tile_pool(name=f"{name}_psb", bufs=2 if pbufs else 1,
                     space="PSUM") as ppsb,
        tc.tile_pool(name=f"{name}_pse", bufs=1, space="PSUM") as ppse,
    ):
        for g0, gn in groups:
            ni_q = gn * qch * P
            sidx = pidx.tile([P, NQ * ni_q // 16], I16, tag="sidx")
            dloc = pidx.tile([P, gn * ch], BF16, tag="dloc")
            nc.sync.dma_start(out=sidx[:, :],
                              in_=srcq[:, scol:scol + NQ * ni_q // 16])
            nc.sync.dma_start(out=dloc[:, :],
                              in_=dlocT[:, g0 * ch:(g0 + gn) * ch])
            scol += NQ * ni_q // 16

            hg = pg.tile([P, NQ, gn, qch, tcols], BF16, tag="hg")
            if "srcg" not in skip:
                for q in range(NQ):
                    _dma_gather(
                        nc.gpsimd,
                        hg[:, q, :, :, :].rearrange("p a b c -> p (a b) c"),
                        tables[q][offs[q]:offs[q] + qsize, 0:tcols],
                        sidx[:, q * ni_q // 16:(q + 1) * ni_q // 16],
                        ni_q, tcols, tab_step, queue_num=q % nsq)
            else:
                w = min(P, tcols)
                nc.vector.tensor_copy(out=hg[:, 0, 0, 0, 0:w],
                                      in_=iota_sb[:, 0:w])

            # Pass 1: gather-INDEPENDENT per-tile work (one-hots, adE, self
            # rows, w_self).  Emitted for the whole group before any
            # gather-dependent op so the in-order engine queues keep busy
            # while the gathers (and, at layer starts, phase A / the
            # AllGather) are still in flight.
            p1 = []
            for ti in range(gn):
                tid = g0 + ti
                rows = last_rows if tid == n_tiles - 1 else P
                dlt = dloc[:, :].rearrange(
                    "p (q a b) -> p q a b", q=NQ, a=gn)[:, :, ti, :]

                mt = pm.tile([P, ch, P], BF16, tag="mt")
                if "vec" not in skip:
                    iota_ap = iota_sb[:, :]
                    iota_v = bass.AP(
                        iota_ap.tensor, iota_ap.offset,
                        [list(iota_ap.ap[0]), [0, NQ], [0, qch], [1, P]])
                    # gather-independent, so safe on the Pool engine (shares
                    # the queue with desc-gen but never stalls it for long)
                    eng = nc.gpsimd if gmt else nc.vector
                    eng.tensor_tensor(
                        out=mt[:, :, :].rearrange("p (q a) b -> p q a b", q=NQ),
                        in0=dlt.to_broadcast([P, NQ, qch, P]),
                        in1=iota_v, op=mybir.AluOpType.is_equal)
                else:
                    nc.vector.memset(mt[:, 0, :], 0.0)

                # per-edge a_d without PE transposes: broadcast the flat
                # dstloc stream across partitions via a K=1 matmul, build the
                # TRANSPOSED one-hot mtT[slot, e] = (dloc[e] == partition)
                # with a per-partition iota scalar, then
                # adE[e, h] = sum_slot mtT[slot, e] * adC[slot, tid, h].
                adE = ppsa.tile([P, ch, heads], F32, tag="adE", space="PSUM")
                if "adg" not in skip:
                    dlf = pidx.tile([1, ch * P], BF16, tag="dlf")
                    nc.sync.dma_start(
                        out=dlf[:, :],
                        in_=dlocF[:, tid * ch * P:(tid + 1) * ch * P])
                    mtT = pm.tile([P, ch * P], BF16, tag="mtT")
                    if pbc:
                        dlrep = pm.tile([P, ch * P], BF16, tag="dlrep")
                        nc.gpsimd.partition_broadcast(dlrep[:, :], dlf[:, :])
                        nc.vector.tensor_scalar(
                            out=mtT[:, :], in0=dlrep[:, :],
                            scalar1=iotac_sb[:, 0:1], scalar2=None,
                            op0=mybir.AluOpType.is_equal)
                    else:
                        for b in range(n_bc):
                            c0 = b * BC
                            cw = min(BC, ch * P - c0)
                            dlr = ppsb.tile([P, BC], F32, tag="dlr",
                                            space="PSUM")
                            nc.tensor.matmul(
                                dlr[:, 0:cw], lhsT=ones_sb[:, :],
                                rhs=dlf[:, c0:c0 + cw],
                                start=True, stop=True)
                            nc.vector.tensor_scalar(
                                out=mtT[:, c0:c0 + cw], in0=dlr[:, 0:cw],
                                scalar1=iotac_sb[:, 0:1], scalar2=None,
                                op0=mybir.AluOpType.is_equal)
                    for k in range(ch):
                        nc.tensor.matmul(adE[:, k, :],
                                         lhsT=mtT[:, k * P:(k + 1) * P],
                                         rhs=adC[:, tid, heads:2 * heads],
                                         start=True, stop=True)
                else:
                    nc.vector.memset(adE[:, :, :], 0.0)

                # self-loop contribution, core-local: h rows of the tile's
                # own dsts re-read contiguously, w_self from captured logits
                hs = psm.tile([P, hdim], BF16, tag="hself")
                nc.sync.dma_start(
                    out=hs[0:rows, :],
                    in_=selftab[tid * P:tid * P + rows, 0:hdim])
                wsA = psm.tile([P, heads], F32, tag="wsA")
                nc.vector.tensor_tensor(
                    out=wsA[:, :], in0=adC[:, tid, 0:heads],
                    in1=adC[:, tid, heads:2 * heads], op=mybir.AluOpType.add)
                nc.vector.scalar_tensor_tensor(
                    out=wsA[:, :], in0=wsA[:, :], scalar=NEG_SLOPE,
                    in1=wsA[:, :], op0=mybir.AluOpType.mult,
                    op1=mybir.AluOpType.max)
                wself = psm.tile([P, heads], F32, tag="wself")
                nc.scalar.activation(out=wself[:, :], in_=wsA[:, :],
                                     func=mybir.ActivationFunctionType.Exp)
                p1.append((mt, adE, hs, wself))

            # Pass 2: gather-dependent softmax, scaling, aggregation, epilogue
            for ti in range(gn):
                tid = g0 + ti
                rows = last_rows if tid == n_tiles - 1 else P
                ht = hg[:, :, ti, :, :]          # [P, NQ, qch, tcols]
                mt, adE, hs, wself = p1[ti]

                sE = psm.tile([P, NQ, qch, heads], F32, tag="sE")
                lrE = psm.tile([P, NQ, qch, heads], F32, tag="lrE")
                if "vec" not in skip:
                    nc.vector.tensor_tensor(
                        out=sE[:, :, :, :],
                        in0=ht[:, :, :, as_off:as_off + heads],
                        in1=adE[:, :, :].rearrange(
                            "p (q a) h -> p q a h", q=NQ),
                        op=mybir.AluOpType.add)
                    nc.vector.scalar_tensor_tensor(
                        out=lrE[:, :, :, :], in0=sE[:, :, :, :],
                        scalar=NEG_SLOPE, in1=sE[:, :, :, :],
                        op0=mybir.AluOpType.mult, op1=mybir.AluOpType.max)
                    nc.scalar.activation(
                        out=ht[:, :, :, as_off:as_off + heads],
                        in_=lrE[:, :, :, :],
                        func=mybir.ActivationFunctionType.Exp)

                    for q in range(NQ):
                        hv = ht[:, q, :, 0:hdim].rearrange(
                            "p a (h c) -> p a h c", h=heads)
                        wv = ht[:, q, :, as_off:as_off + heads].to_broadcast(
                            [P, qch, heads, hdim // heads])
                        nc.vector.tensor_tensor(out=hv, in0=hv, in1=wv,
                                                op=mybir.AluOpType.mult)

                ps = pps.tile([P, rcols], F32, tag="agg", space="PSUM")
                if "mm" not in skip:
                    for k in range(ch):
                        nc.tensor.matmul(ps[:, :], lhsT=mt[:, k, :],
                                         rhs=ht[:, k // qch, k % qch, 0:rcols],
                                         start=(k == 0), stop=(k == ch - 1))
                else:
                    nc.vector.memset(ps[:, :], 0.0)

                # denominators + self weight, numerators + wself*h_self
                den = psm.tile([P, heads], F32, tag="den")
                nc.vector.tensor_tensor(
                    out=den[:, :], in0=ps[:, hdim:hdim + heads],
                    in1=wself[:, :], op=mybir.AluOpType.add)
                rec = psm.tile([P, heads], F32, tag="rec")
                nc.vector.reciprocal(rec[:, :], den[:, :])
                cph = hdim // heads
                num = psm.tile([P, hdim], F32, tag="num")
                for h in range(heads):
                    nc.vector.scalar_tensor_tensor(
                        out=num[:, h * cph:(h + 1) * cph],
                        in0=hs[:, h * cph:(h + 1) * cph],
                        scalar=wself[:, h:h + 1],
                        in1=ps[:, h * cph:(h + 1) * cph],
                        op0=mybir.AluOpType.mult, op1=mybir.AluOpType.add)

                if layer == 1:
                    t0 = psm.tile([P, HIDDEN], F32, tag="t0")
                    nc.vector.tensor_scalar(
                        out=t0[:, :], in0=num[:, 0:HIDDEN],
                        scalar1=rec[:, 0:1], scalar2=None,
                        op0=mybir.AluOpType.mult)
                    nc.vector.scalar_tensor_tensor(
                        out=t0[:, :], in0=num[:, HIDDEN:2 * HIDDEN],
                        scalar=rec[:, 1:2], in1=t0[:, :],
                        op0=mybir.AluOpType.mult, op1=mybir.AluOpType.add)
                    hb = psm.tile([P, HIDDEN], F32, tag="hb")
                    nc.vector.scalar_tensor_tensor(
                        out=hb[:, :], in0=t0[:, :], scalar=0.5, in1=b1_sb[:, :],
                        op0=mybir.AluOpType.mult, op1=mybir.AluOpType.add)
                    hr = psm.tile([P, HIDDEN], BF16, tag="hr")
                    nc.scalar.activation(out=hr[:, :], in_=hb[:, :],
                                         func=mybir.ActivationFunctionType.Relu)
                    psT = ppse.tile([P, P], BF16, tag="psT", space="PSUM")
                    nc.tensor.transpose(out=psT[:, :], in_=hr[:, :],
                                        identity=id_sb[:, :])
                    hrT = psm.tile([P, P], BF16, tag="hrT")
                    nc.scalar.copy(out=hrT[:, :], in_=psT[:, :])
                    ps2 = ppse.tile([P, OUT_DIM + 2], F32, tag="ps2",
                                    space="PSUM")
                    nc.tensor.matmul(ps2[:, :], lhsT=hrT[:, :], rhs=w2_sb[:, :],
                                     start=True, stop=True)
                    t2 = psm.tile([P, OUT_DIM + 2], BF16, tag="t2")
                    nc.vector.tensor_copy(out=t2[:, :], in_=ps2[:, :])
                    nc.scalar.copy(out=adC_next[0:rows, tid, :],
                                   in_=t2[0:rows, OUT_DIM:OUT_DIM + 2])
                    nc.sync.dma_start(
                        out=t2shard[tid * P:tid * P + rows, 0:OUT_DIM + 2],
                        in_=t2[0:rows, :])
                    if debug_out:
                        dbg = psm.tile([P, OUT_DIM], F32, tag="dbg")
                        nc.vector.tensor_copy(out=dbg[:, :],
                                              in_=ps2[:, 0:OUT_DIM])
                        nc.sync.dma_start(
                            out=out_shard[tid * P:tid * P + rows, :],
                            in_=dbg[0:rows, :])
                else:
                    of = psm.tile([P, OUT_DIM], F32, tag="of")
                    nc.vector.tensor_scalar(
                        out=of[:, :], in0=num[:, 0:OUT_DIM],
                        scalar1=rec[:, 0:1], scalar2=None,
                        op0=mybir.AluOpType.mult)
                    nc.vector.tensor_tensor(
                        out=of[:, :], in0=of[:, :], in1=b2_sb[:, :],
                        op=mybir.AluOpType.add)
                    nc.sync.dma_start(
                        out=out_shard[tid * P:tid * P + rows, :],
                        in_=of[0:rows, :])


# ================================================================ entry point
def kernel(**inputs):
    cfg = dict(FULL_CFG)
    cfg["n"] = N
    in_maps, meta = _host_inputs(
        inputs["x"], inputs["edge_index"], inputs["W1"], inputs["att_src1"],
        inputs["att_dst1"], inputs["b1"], inputs["W2"], inputs["att_src2"],
        inputs["att_dst2"], inputs["b2"], cfg)
    nc = build_program(cfg, meta)
    # transient device wedges (NRT_EXEC_UNIT_UNRECOVERABLE) self-heal after a
    # few minutes; retry rather than failing the whole run
    import time as _time
    last = None
    for attempt in range(4):
        try:
            res = run_bass_kernel_spmd(
                nc, in_maps, core_ids=list(range(cfg["n_cores"])))
            break
        except Exception as exc:  # noqa: BLE001
            last = exc
            if attempt == 3:
                raise
            _time.sleep(90)
    out = unpermute(meta, cfg["shard"],
                    [res.results[c]["out_shard"]
                     for c in range(cfg["n_cores"])])
    return out.astype(np.float32)

